# revision 1
# baseline (speedup 1.0000x reference)
"""Differential attention (B=2, S=2048, D=2048, H=16) on 8 Trainium2 cores.

Sharding: core c -> batch b=c//4, head group hg=c%4 (4 heads each).
Each core computes qkv projection for its head columns, RoPE, differential
attention, per-head LayerNorm, and a partial @W_o over its 512 vd rows.
Host sums the 4 partials per batch. No duplicated FLOPs, no collectives.

Layout tricks:
 - qkv computed transposed ([dims, tokens]) with host-permuted W columns so
   RoPE even/odd pairs become contiguous 128-partition blocks.
 - attention scores computed transposed ([k, q]) so exp -> v.T @ e accumulates
   the attention output directly in [vd, q] layout for the W_o matmul.
 - softmax denominators + LN stats via ones-vector matmuls (M=1 / column sums).
 - fp32r for all precision-bearing matmuls (full PE speed at N=512, ~1e-4 err).
"""
import sys

sys.path.insert(0, "/opt/trn_rl_repo")

import numpy as np

B, S, D = 2, 2048, 2048
H = 16
HD = D // (2 * H)          # 64 per-map head dim
DH = 2 * HD                # 128 per-head dim
HPC = H // 4               # 4 heads per core
NCORES = 8
SCALE = HD ** -0.5         # 0.125
NEG = -8.0e9               # mask add value pre-scale (-1e9 / SCALE)
OUT_MULT = 1.0 - 0.8       # (1 - LBDA_INIT)

# module-level knobs / results for test.py
TRACE = False
TRACE_DIR = None
LAST_RESULTS = None
LAST_EXEC_NS = None

_PROGRAM_CACHE = {}


def build_program(s=S):
    """Build the per-core Bass program (SPMD: same program, 8 cores)."""
    import concourse.bass as bass
    import concourse.tile as tile
    from concourse import bacc, mybir
    from concourse.bass import ts, ds

    f32 = mybir.dt.float32
    f32r = mybir.dt.float32r
    AF = mybir.ActivationFunctionType
    OP = mybir.AluOpType

    NCH = s // 512              # token chunks of 512
    KT = s // 128               # k tiles of 128
    KO = D // 128               # contraction chunks over D

    nc = bacc.Bacc()
    xT = nc.declare_dram_parameter("xT", [D, s], f32, isOutput=False)
    wqk = nc.declare_dram_parameter("wqk", [D, 8 * 128], f32, isOutput=False)
    wv = nc.declare_dram_parameter("wv", [D, HPC * DH], f32, isOutput=False)
    wo = nc.declare_dram_parameter("wo", [HPC * DH, D], f32, isOutput=False)
    cs = nc.declare_dram_parameter("cs", [128, s], f32, isOutput=False)
    sn = nc.declare_dram_parameter("sn", [128, s], f32, isOutput=False)
    gb = nc.declare_dram_parameter("gb", [128, 2 * HPC], f32, isOutput=False)
    mw = nc.declare_dram_parameter("mw", [128, 896], f32, isOutput=False)
    lam = nc.declare_dram_parameter("lam", [128, 1], f32, isOutput=False)
    onec = nc.declare_dram_parameter("onec", [128, 128], f32, isOutput=False)
    onel = nc.declare_dram_parameter("onel", [128, 128], f32, isOutput=False)
    out = nc.declare_dram_parameter("out", [s, D], f32, isOutput=True)

    SQ = nc.dram_tensor("SQ", [HPC * DH, s], f32)
    SK = nc.dram_tensor("SK", [HPC * DH, s], f32)
    SV = nc.dram_tensor("SV", [s, HPC * DH], f32)

    r = lambda ap: ap.bitcast(f32r)

    with tile.TileContext(nc) as tc:
        # ---------------- Phase 1: qkvT projection + RoPE -> DRAM scratch ----
        with tc.tile_pool(name="p1w", bufs=1) as p1w, \
             tc.tile_pool(name="p1x", bufs=2) as p1x, \
             tc.tile_pool(name="p1o", bufs=3) as p1o, \
             tc.tile_pool(name="p1t", bufs=2) as p1t, \
             tc.tile_pool(name="p1ps", bufs=4, space="PSUM") as p1ps:
            wqk_sb = p1w.tile([128, KO, 8 * 128], f32r)
            wv_sb = p1w.tile([128, KO, HPC * DH], f32r)
            for ko in range(KO):
                nc.gpsimd.dma_start(wqk_sb[:, ko, :], r(wqk[ds(ko * 128, 128), :]))
                nc.gpsimd.dma_start(wv_sb[:, ko, :], r(wv[ds(ko * 128, 128), :]))
            cs_sb = p1w.tile([128, s], f32)
            nc.gpsimd.dma_start(cs_sb[:], cs[:])
            sn_sb = p1w.tile([128, s], f32)
            nc.gpsimd.dma_start(sn_sb[:], sn[:])

            def emit_v_pass(ncI, xt):
                # v in [token, vd] orientation
                for tsub in range(4):
                    pv = p1ps.tile([128, HPC * DH], f32, tag="p1b")
                    for ko in range(KO):
                        nc.tensor.matmul(pv[:], xt[:, ko, ts(tsub, 128)],
                                         wv_sb[:, ko, :], start=(ko == 0),
                                         stop=(ko == KO - 1))
                    ov = p1o.tile([128, HPC * DH], f32, tag="ov")
                    nc.any.tensor_copy(out=ov[:], in_=pv[:])
                    nc.gpsimd.dma_start(SV[ds(ncI * 512 + tsub * 128, 128), :], ov[:])

            xT3 = xT.rearrange("(ko p) t -> p ko t", p=128)
            SQ3 = SQ.rearrange("(h d) t -> h d t", h=HPC)
            SK3 = SK.rearrange("(h d) t -> h d t", h=HPC)

            for ncI in range(NCH):
                xt = p1x.tile([128, KO, 512], f32r)
                for ko in range(KO):
                    nc.sync.dma_start(xt[:, ko, :],
                                      r(xT[ds(ko * 128, 128), ts(ncI, 512)]))
                csc = cs_sb[:, ts(ncI, 512)]
                snc = sn_sb[:, ts(ncI, 512)]
                # 1a: q/k blocks, RoPE'd in pairs (even, odd); k first,
                # then v (between), q last -- phase 2's kt/vt loads gate on these
                for pair in (2, 3, 0, 1):       # k1, k2, q1, q2
                    if pair == 0:
                        emit_v_pass(ncI, xt)
                    be, bo = 2 * pair, 2 * pair + 1
                    pe_ = p1ps.tile([128, 512], f32, tag="p1a")
                    po_ = p1ps.tile([128, 512], f32, tag="p1a")
                    for ko in range(KO):
                        nc.tensor.matmul(pe_[:], wqk_sb[:, ko, ts(be, 128)],
                                         xt[:, ko, :], start=(ko == 0), stop=(ko == KO - 1))
                    for ko in range(KO):
                        nc.tensor.matmul(po_[:], wqk_sb[:, ko, ts(bo, 128)],
                                         xt[:, ko, :], start=(ko == 0), stop=(ko == KO - 1))
                    t1 = p1t.tile([128, 512], f32, tag="t1")
                    t2 = p1t.tile([128, 512], f32, tag="t2")
                    oe = p1o.tile([128, 512], f32, tag="oe")
                    oo = p1o.tile([128, 512], f32, tag="oo")
                    nc.vector.tensor_tensor(t1[:], pe_[:], csc, OP.mult)
                    nc.vector.tensor_tensor(t2[:], po_[:], snc, OP.mult)
                    nc.vector.tensor_tensor(oe[:], t1[:], t2[:], OP.subtract)
                    nc.vector.tensor_tensor(t1[:], pe_[:], snc, OP.mult)
                    nc.vector.tensor_tensor(t2[:], po_[:], csc, OP.mult)
                    nc.vector.tensor_tensor(oo[:], t1[:], t2[:], OP.add)
                    dst = SQ if pair < 2 else SK
                    moff = (pair % 2) * 64
                    for hh in range(HPC):
                        nc.sync.dma_start(
                            dst[ds(hh * DH + moff, 32), ts(ncI, 512)],
                            oe[ds(hh * 32, 32), :])
                        nc.gpsimd.dma_start(
                            dst[ds(hh * DH + moff + 32, 32), ts(ncI, 512)],
                            oo[ds(hh * 32, 32), :])

        # ---------------- Phase 2: differential attention + LN ---------------
        with tc.tile_pool(name="attnp", bufs=1) as attnp, \
             tc.tile_pool(name="p3w", bufs=1) as p3w:
          attn_sb = attnp.tile([128, HPC, s], f32r)
          wo_sb = p3w.tile([128, HPC, D], f32r)
          for hh in range(HPC):
              nc.gpsimd.dma_start(wo_sb[:, hh, :], r(wo[ds(hh * 128, 128), :]))
          with tc.tile_pool(name="p2c", bufs=1) as p2c, \
             tc.tile_pool(name="p2kv", bufs=2) as p2kv, \
             tc.tile_pool(name="p2q", bufs=2) as p2q, \
             tc.tile_pool(name="p2e", bufs=8) as p2e, \
             tc.tile_pool(name="p2t", bufs=4) as p2t, \
             tc.tile_pool(name="p2st", bufs=12) as p2st, \
             tc.tile_pool(name="p2pp", bufs=8, space="PSUM") as p2pp:
            mw_sb = p2c.tile([128, 896], f32)
            nc.sync.dma_start(mw_sb[:], mw[:])
            gb_sb = p2c.tile([128, 2 * HPC], f32)
            nc.sync.dma_start(gb_sb[:], gb[:])
            lam_sb = p2c.tile([128, 1], f32)
            nc.sync.dma_start(lam_sb[:], lam[:])
            ones_c = p2c.tile([128, 128], f32r)
            nc.sync.dma_start(ones_c[:], r(onec[:]))
            ones_l = p2c.tile([128, 128], f32r)
            nc.sync.dma_start(ones_l[:], r(onel[:]))
            eps_sb = p2c.tile([1, 1], f32)
            nc.vector.memset(eps_sb[:], 1e-5)
            mu_all = p2c.tile([128, 512], f32)
            var_all = p2c.tile([128, 512], f32)

            SV4 = SV.rearrange("(k p) (hh d) -> p k hh d", p=128, hh=HPC)

            def emit_stats(h, qc):
                it = qc * 32 + h
                att = attn_sb[:, h, ts(qc, 512)]
                sq = p2t.tile([128, 512], f32r, tag="sq")
                nc.vector.tensor_tensor(sq[:], att, att, OP.mult)
                MSm = p2pp.tile([128, 512], f32, tag="pp")
                MSs = p2pp.tile([128, 512], f32, tag="pp")
                nc.tensor.matmul(MSm[:], ones_c[:], att, start=True, stop=True,
                                 skip_group_check=True)
                nc.tensor.matmul(MSs[:], ones_c[:], sq[:], start=True, stop=True,
                                 skip_group_check=True)
                ex2 = p2st.tile([1, 512], f32, tag="st")
                musq = p2st.tile([1, 512], f32, tag="st")
                mu = p2st.tile([1, 512], f32, tag="st")
                var = p2st.tile([1, 512], f32, tag="st")
                nc.vector.tensor_scalar_mul(mu[:], MSm[0:1, :], 1.0 / DH)
                nc.vector.tensor_scalar_mul(ex2[:], MSs[0:1, :], 1.0 / DH)
                nc.vector.tensor_tensor(musq[:], mu[:], mu[:], OP.mult)
                nc.vector.tensor_tensor(var[:], ex2[:], musq[:], OP.subtract)
                nc.gpsimd.dma_start(mu_all[it:it + 1, :], mu[:])
                nc.gpsimd.dma_start(var_all[it:it + 1, :], var[:])

            sd_all = p2c.tile([128, 512], f32)
            rstd_all = p2c.tile([128, 512], f32)
            murs_all = p2c.tile([128, 512], f32)
            eps16 = p2c.tile([128, 1], f32)
            nc.vector.memset(eps16[:], 1e-5)

            def emit_norm(qc):
                rows = ds(qc * 32, HPC)
                nc.scalar.activation(sd_all[rows, :], var_all[rows, :], AF.Sqrt,
                                     bias=eps16[rows, :])
                nc.vector.reciprocal(out=rstd_all[rows, :], in_=sd_all[rows, :])
                nc.vector.tensor_tensor(murs_all[rows, :], mu_all[rows, :],
                                        rstd_all[rows, :], OP.mult)
                for h in range(HPC):
                    it = qc * 32 + h
                    att = attn_sb[:, h, ts(qc, 512)]
                    r3s = p2t.tile([128, 512], f32, tag="r1s")
                    r4s = p2t.tile([128, 512], f32, tag="r2s")
                    nc.gpsimd.dma_start(
                        out=r3s[:],
                        in_=rstd_all[it:it + 1, :].unsqueeze(1).to_broadcast([1, 128, 512]))
                    nc.gpsimd.dma_start(
                        out=r4s[:],
                        in_=murs_all[it:it + 1, :].unsqueeze(1).to_broadcast([1, 128, 512]))
                    t1 = p2t.tile([128, 512], f32, tag="t1")
                    nc.vector.tensor_tensor(t1[:], att, r3s[:], OP.mult)
                    nc.vector.tensor_tensor(t1[:], t1[:], r4s[:], OP.subtract)
                    nc.scalar.activation(att, t1[:], AF.Identity,
                                         bias=gb_sb[:, HPC + h:HPC + h + 1],
                                         scale=gb_sb[:, h:h + 1])

            iters = [(h, qc) for h in range(HPC) for qc in range(NCH)]
            kts, vts, qts = {}, {}, {}

            def load_qt(idx2):
                h2, qc2 = iters[idx2]
                qts[idx2] = p2q.tile([128, 512], f32r, name=f"qt{idx2}", tag="qt")
                nc.sync.dma_start(qts[idx2][:],
                                  r(SQ[ds(h2 * DH, DH), ts(qc2, 512)]))

            def load_head(hh):
                kts[hh] = p2kv.tile([128, s], f32r, tag="kt", name=f"kt{hh}")
                nc.sync.dma_start(kts[hh][:], r(SK[ds(hh * DH, DH), :]))
                vts[hh] = p2kv.tile([128, KT, DH], f32r, tag="vt", name=f"vt{hh}")
                nc.gpsimd.dma_start(vts[hh][:], r(SV4[:, :, hh, :]))

            load_head(0)
            load_qt(0)
            for idx, (h, qc) in enumerate(iters):
                kt, vt = kts[h], vts[h]
                if qc == 0 and h + 1 < HPC:
                    load_head(h + 1)
                if True:
                    qt = qts.pop(idx)
                    U1 = p2pp.tile([128, 512], f32, tag="pp")
                    U2 = p2pp.tile([128, 512], f32, tag="pp")
                    D1 = p2pp.tile([128, 512], f32, tag="pp")
                    D2 = p2pp.tile([128, 512], f32, tag="pp")
                    klim = 4 * qc + 4

                    def emit_scores(ki):
                        p = ki - 4 * qc
                        # columns left of the causal diagonal are fully masked:
                        # skip them in scores/exp/av/dsum entirely
                        c0 = 128 * p if p > 0 else 0
                        s1 = p2pp.tile([128, 512], f32, tag="pp", name=f"s1_{ki}")
                        s2 = p2pp.tile([128, 512], f32, tag="pp", name=f"s2_{ki}")
                        nc.tensor.matmul(s1[:, c0:], kt[0:64, ts(ki, 128)],
                                         qt[0:64, c0:], start=True, stop=True)
                        nc.tensor.matmul(s2[:, c0:], kt[64:128, ts(ki, 128)],
                                         qt[64:128, c0:], start=True, stop=True)
                        if p >= 0:  # mask only the 128-wide diagonal block
                            dg = mw_sb[:, 384:512]
                            nc.vector.tensor_tensor(s1[:, c0:c0 + 128],
                                                    s1[:, c0:c0 + 128], dg, OP.add)
                            nc.vector.tensor_tensor(s2[:, c0:c0 + 128],
                                                    s2[:, c0:c0 + 128], dg, OP.add)
                        e1 = p2e.tile([128, 512], f32r, tag="e", name=f"e1_{ki}")
                        e2 = p2e.tile([128, 512], f32r, tag="e", name=f"e2_{ki}")
                        nc.scalar.activation(e1[:, c0:], s1[:, c0:], AF.Exp, scale=SCALE)
                        nc.scalar.activation(e2[:, c0:], s2[:, c0:], AF.Exp, scale=SCALE)
                        return e1, e2, c0

                    # scores/exp emitted one k-tile ahead so the PE's av/dsum
                    # group never waits on ACT's exp of the same tile
                    es = {0: emit_scores(0)}
                    for ki in range(klim):
                        if ki + 1 < klim:
                            es[ki + 1] = emit_scores(ki + 1)
                        e1, e2, c0 = es.pop(ki)
                        st, sp = (ki == 0), (ki == klim - 1)
                        nc.tensor.matmul(U1[:, c0:], vt[:, ki, :], e1[:, c0:],
                                         start=st, stop=sp)
                        nc.tensor.matmul(U2[:, c0:], vt[:, ki, :], e2[:, c0:],
                                         start=st, stop=sp)
                        nc.tensor.matmul(D1[:, c0:], ones_c[:], e1[:, c0:],
                                         start=st, stop=sp, skip_group_check=True)
                        nc.tensor.matmul(D2[:, c0:], ones_l[:], e2[:, c0:],
                                         start=st, stop=sp, skip_group_check=True)
                    if idx + 1 < len(iters):
                        load_qt(idx + 1)
                    # epilogue: D came out replicated across partitions,
                    # so normalize directly (no broadcast step needed)
                    r1s = p2t.tile([128, 512], f32, tag="r1s")
                    r2s = p2t.tile([128, 512], f32, tag="r2s")
                    nc.vector.reciprocal(out=r1s[:], in_=D1[:])
                    nc.vector.reciprocal(out=r2s[:], in_=D2[:])
                    t1 = p2t.tile([128, 512], f32, tag="t1")
                    t2 = p2t.tile([128, 512], f32, tag="t2")
                    att = attn_sb[:, h, ts(qc, 512)]
                    nc.vector.tensor_tensor(t1[:], U1[:], r1s[:], OP.mult)
                    nc.vector.tensor_tensor(t2[:], U2[:], r2s[:], OP.mult)
                    nc.vector.tensor_tensor(att, t1[:], t2[:], OP.subtract)
                    if idx >= 1:
                        h2, qc2 = iters[idx - 1]
                        emit_stats(h2, qc2)
                        if h2 == HPC - 1:
                            emit_norm(qc2)
            for h2, qc2 in iters[-1:]:
                emit_stats(h2, qc2)
                if h2 == HPC - 1:
                    emit_norm(qc2)


          # ------------- Phase 3: partial @ W_o ------------------------------
          with tc.tile_pool(name="p3o", bufs=4) as p3o, \
               tc.tile_pool(name="p3ps", bufs=6, space="PSUM") as p3ps:
              for qi in range(s // 128):
                  for nj in range(D // 512):
                      po = p3ps.tile([128, 512], f32)
                      for h in range(HPC):
                          nc.tensor.matmul(po[:], attn_sb[:, h, ts(qi, 128)],
                                           wo_sb[:, h, ts(nj, 512)],
                                           start=(h == 0), stop=(h == HPC - 1))
                      oo = p3o.tile([128, 512], f32)
                      nc.any.tensor_copy(out=oo[:], in_=po[:])
                      nc.gpsimd.dma_start(out[ts(qi, 128), ts(nj, 512)], oo[:])

    nc.finalize()
    return nc


def get_program(s=S):
    if s not in _PROGRAM_CACHE:
        _PROGRAM_CACHE[s] = build_program(s)
    return _PROGRAM_CACHE[s]


def make_core_inputs(x, cos, sin, W_qkv, W_o, ln_gamma, ln_beta, lbda, core, s=S):
    """Host-side shard prep for one core."""
    b, hg = core // 4, core % 4
    heads = list(range(hg * HPC, (hg + 1) * HPC))

    def qk_block_cols(base, dstart):
        # even/odd pair columns for one 32-wide block across the 4 heads
        return [base + hh * DH + dstart + 2 * p for hh in heads for p in range(32)]

    cols = []
    for base in (0, D):                       # q section, k section
        for dstart in (0, 1, HD, HD + 1):     # m1-even, m1-odd, m2-even, m2-odd
            cols += qk_block_cols(base, dstart)
    wqk = np.ascontiguousarray(W_qkv[:, cols], dtype=np.float32)
    vcols = [2 * D + hh * DH + dd for hh in heads for dd in range(DH)]
    wv = np.ascontiguousarray(W_qkv[:, vcols], dtype=np.float32)
    worows = [hh * DH + dd for hh in heads for dd in range(DH)]
    wo = np.ascontiguousarray(W_o[worows, :], dtype=np.float32)

    xT = np.ascontiguousarray(x[b].T, dtype=np.float32)
    cs = np.ascontiguousarray(np.tile(cos.T, (HPC, 1)), dtype=np.float32)
    sn = np.ascontiguousarray(np.tile(sin.T, (HPC, 1)), dtype=np.float32)

    gb = np.zeros((128, 2 * HPC), dtype=np.float32)
    for j, hh in enumerate(heads):
        gb[:, j] = ln_gamma[hh] * OUT_MULT
        gb[:, HPC + j] = ln_beta[hh] * OUT_MULT

    mwide = np.zeros((128, 896), dtype=np.float32)
    mwide[:, :384] = NEG
    diag = np.where(np.triu(np.ones((128, 128), dtype=bool)), 0.0, NEG)
    mwide[:, 384:512] = diag.astype(np.float32)

    return {
        "xT": xT, "wqk": wqk, "wv": wv, "wo": wo, "cs": cs, "sn": sn,
        "gb": gb, "mw": mwide,
        "lam": np.full((128, 1), lbda, dtype=np.float32),
        "onec": np.ones((128, 128), dtype=np.float32),
        "onel": np.full((128, 128), 1.0 / lbda if lbda != 0 else 1e30,
                        dtype=np.float32),
    }


def _mask_is_causal(mask, s=S):
    m = np.asarray(mask).reshape(s, s)
    tril = np.tril(np.ones((s, s), dtype=bool))
    if not np.array_equal(m == 0.0, tril):
        return False
    off = m[~tril]
    return off.size == 0 or (np.all(off <= -1.0e8) and np.all(np.isfinite(off)))


def _numpy_reference(x, mask, cos, sin, W_qkv, W_o, ln_gamma, ln_beta, lbda):
    """Exact-math fallback (used only if the mask is not the causal pattern)."""
    b, s, d = x.shape
    qkv = x @ W_qkv
    q, k, v = np.split(qkv, 3, axis=-1)
    q = q.reshape(b, s, H, DH).transpose(0, 2, 1, 3)
    k = k.reshape(b, s, H, DH).transpose(0, 2, 1, 3)
    v = v.reshape(b, s, H, DH).transpose(0, 2, 1, 3)

    def rope(t):
        tr = t.reshape(b, H, s, HD // 2, 2)
        x1, x2 = tr[..., 0], tr[..., 1]
        c = cos[None, None]
        sn_ = sin[None, None]
        o1 = x1 * c - x2 * sn_
        o2 = x1 * sn_ + x2 * c
        return np.stack([o1, o2], axis=-1).reshape(b, H, s, HD)

    q1, q2 = q[..., :HD], q[..., HD:]
    k1, k2 = k[..., :HD], k[..., HD:]
    q1, k1 = rope(q1), rope(k1)
    q2, k2 = rope(q2), rope(k2)

    def softm(z):
        z = z - z.max(-1, keepdims=True)
        e = np.exp(z)
        return e / e.sum(-1, keepdims=True)

    m = np.asarray(mask).reshape(1, 1, s, s)
    a1 = softm(np.einsum("bhqd,bhkd->bhqk", q1, k1) * SCALE + m)
    a2 = softm(np.einsum("bhqd,bhkd->bhqk", q2, k2) * SCALE + m)
    a = a1 - float(lbda) * a2
    o = np.einsum("bhqk,bhkd->bhqd", a, v)
    mu = o.mean(-1, keepdims=True)
    var = o.var(-1, keepdims=True)
    o = (o - mu) / np.sqrt(var + 1e-5)
    o = o * ln_gamma[None, :, None, :] + ln_beta[None, :, None, :]
    o = o * OUT_MULT
    o = o.transpose(0, 2, 1, 3).reshape(b, s, d)
    return (o @ W_o).astype(np.float32)


def kernel(x, mask, cos, sin, W_qkv, W_o, ln_gamma, ln_beta, lbda):
    global LAST_RESULTS, LAST_EXEC_NS
    x = np.asarray(x, dtype=np.float32)
    cos = np.asarray(cos, dtype=np.float32)
    sin = np.asarray(sin, dtype=np.float32)
    W_qkv = np.asarray(W_qkv, dtype=np.float32)
    W_o = np.asarray(W_o, dtype=np.float32)
    ln_gamma = np.asarray(ln_gamma, dtype=np.float32)
    ln_beta = np.asarray(ln_beta, dtype=np.float32)
    lbda_f = float(np.asarray(lbda))

    if not _mask_is_causal(mask):
        return _numpy_reference(x, mask, cos, sin, W_qkv, W_o,
                                ln_gamma, ln_beta, lbda_f)

    from concourse.bass_utils import run_bass_kernel_spmd

    nc = get_program(S)
    in_maps = [
        make_core_inputs(x, cos, sin, W_qkv, W_o, ln_gamma, ln_beta, lbda_f, c)
        for c in range(NCORES)
    ]
    kwargs = {"trace": TRACE}
    if TRACE and TRACE_DIR:
        kwargs["tmpdir"] = TRACE_DIR
    res = run_bass_kernel_spmd(nc, in_maps, core_ids=list(range(NCORES)),
                               **kwargs)
    LAST_RESULTS = res
    LAST_EXEC_NS = getattr(res, "exec_time_ns", None)

    outf = np.zeros((B, S, D), dtype=np.float32)
    for c in range(NCORES):
        outf[c // 4] += res.results[c]["out"]
    return outf



# revision 37
# speedup vs baseline: 1.1618x; 1.1618x over previous
"""Differential attention (B=2, S=2048, D=2048, H=16) on 8 Trainium2 cores.

Sharding: core c -> batch b=c//4, head group hg=c%4 (4 heads each).
Each core computes qkv projection for its head columns, RoPE, differential
attention, per-head LayerNorm, and a partial @W_o over its 512 vd rows.
Host sums the 4 partials per batch. No duplicated FLOPs, no collectives.

Pipeline (single pass, no DRAM scratch):
 - window1: per 512-token chunk, project k (2 maps, RoPE'd) and v, scatter
   straight into SBUF-resident kt/vt tiles via SBUF->SBUF DMA.
 - window2: per chunk, project q (RoPE'd, SBUF-resident), then immediately
   run the attention row qc=chunk for all 4 heads (scores transposed [k,q],
   fused two-map PSUM tile + single exp per k-tile, fp32r matmuls),
   LayerNorm stats via replicated ones-matmuls (no broadcast DMAs), and the
   partial @W_o for that chunk row. PE never waits on a phase barrier.
 - diagonal k-tiles padded to >=256 moving columns (fp32r runs 4x slower
   below 256) with a NEG|diag mask block.
"""
import sys

sys.path.insert(0, "/opt/trn_rl_repo")

import numpy as np

B, S, D = 2, 2048, 2048
H = 16
HD = D // (2 * H)          # 64 per-map head dim
DH = 2 * HD                # 128 per-head dim
HPC = H // 4               # 4 heads per core
NCORES = 8
SCALE = HD ** -0.5         # 0.125
NEG = -8.0e9               # mask add value pre-scale (-1e9 / SCALE)
OUT_MULT = 1.0 - 0.8       # (1 - LBDA_INIT)

TRACE = False
TRACE_DIR = None
LAST_RESULTS = None
LAST_EXEC_NS = None

_PROGRAM_CACHE = {}


def build_program(s=S):
    """Build the per-core Bass program (SPMD: same program, 8 cores)."""
    import concourse.bass as bass
    import concourse.tile as tile
    from concourse import bacc, mybir
    from concourse.bass import ts, ds

    f32 = mybir.dt.float32
    f32r = mybir.dt.float32r
    bf16 = mybir.dt.bfloat16
    AF = mybir.ActivationFunctionType
    OP = mybir.AluOpType

    NCH = s // 512              # token chunks of 512
    KT = s // 128               # k tiles of 128
    KO = D // 128               # contraction chunks over D

    nc = bacc.Bacc()
    xT = nc.declare_dram_parameter("xT", [D, s], f32, isOutput=False)
    xTb = nc.declare_dram_parameter("xTb", [D, s], bf16, isOutput=False)
    wk = nc.declare_dram_parameter("wk", [D, 4 * 128], f32, isOutput=False)
    wq = nc.declare_dram_parameter("wq", [D, 4 * 128], f32, isOutput=False)
    wv = nc.declare_dram_parameter("wv", [D, HPC * DH], bf16, isOutput=False)
    wo = nc.declare_dram_parameter("wo", [HPC * DH, D], bf16, isOutput=False)
    cs = nc.declare_dram_parameter("cs", [128, s], f32, isOutput=False)
    sn = nc.declare_dram_parameter("sn", [128, s], f32, isOutput=False)
    gb = nc.declare_dram_parameter("gb", [128, 2 * HPC], f32, isOutput=False)
    mw = nc.declare_dram_parameter("mw", [128, 256], f32, isOutput=False)
    onem = nc.declare_dram_parameter("onem", [128, 128], bf16, isOutput=False)
    onec = nc.declare_dram_parameter("onec", [128, 128], bf16, isOutput=False)
    onel = nc.declare_dram_parameter("onel", [128, 128], bf16, isOutput=False)
    onecr = nc.declare_dram_parameter("onecr", [128, 128], f32, isOutput=False)
    onelr = nc.declare_dram_parameter("onelr", [128, 128], f32, isOutput=False)
    out = nc.declare_dram_parameter("out", [s, D], f32, isOutput=True)

    r = lambda ap: ap.bitcast(f32r)

    with tile.TileContext(nc) as tc:
        with tc.tile_pool(name="pL", bufs=1) as pL, \
             tc.tile_pool(name="pO", bufs=2) as pO, \
             tc.tile_pool(name="pT", bufs=2) as pT, \
             tc.tile_pool(name="pCS", bufs=2) as pCS, \
             tc.tile_pool(name="pp", bufs=4, space="PSUM") as pp:
            # ---- long-lived SBUF state -----------------------------------
            kt = pL.tile([128, HPC, s], bf16)       # per-head k dims x tokens
            qt = pL.tile([128, HPC, s], bf16)       # per-head q dims x tokens
            vt = pL.tile([128, KT, HPC, DH], bf16)  # k-token x (ki, h, vd)
            wv_sb = pL.tile([128, KO, 512], bf16)
            gb_sb = pL.tile([128, 2 * HPC], f32)
            nc.sync.dma_start(gb_sb[:], gb[:])
            mw_sb = pL.tile([128, 2, 128], f32)
            nc.sync.dma_start(mw_sb[:, :, :], mw[:])
            ones_m = pL.tile([128, 128], bf16)
            nc.sync.dma_start(ones_m[:], onem[:])
            ones_c = pL.tile([128, 128], bf16)
            nc.sync.dma_start(ones_c[:], onec[:])
            ones_l = pL.tile([128, 128], bf16)
            nc.sync.dma_start(ones_l[:], onel[:])
            ones_cr = pL.tile([128, 128], f32r)
            nc.sync.dma_start(ones_cr[:], r(onecr[:]))
            ones_lr = pL.tile([128, 128], f32r)
            nc.sync.dma_start(ones_lr[:], r(onelr[:]))

            def load_x(pool, ncI, tagpfx, engs):
                xth = []
                for hf in range(2):
                    eng = engs[hf]
                    xh = pool.tile([128, KO, 256], f32r, tag="xt", bufs=2,
                                   name=f"x{tagpfx}{ncI}_{hf}")
                    for ko in range(KO):
                        eng.dma_start(
                            xh[:, ko, :],
                            r(xT[ds(ko * 128, 128),
                                 ds(ncI * 512 + hf * 256, 256)]))
                    xth.append(xh)
                return xth

            def load_xv(ncI, engs=(None, None)):
                xth = []
                for hf in range(2):
                    eng = engs[hf] or nc.sync
                    xh = pW2.tile([128, KO, 256], bf16, tag="xv", bufs=2,
                                  name=f"xv{ncI}_{hf}")
                    for ko in range(KO):
                        eng.dma_start(
                            xh[:, ko, :],
                            xTb[ds(ko * 128, 128),
                                ds(ncI * 512 + hf * 256, 256)])
                    xth.append(xh)
                return xth

            def load_cs(ncI, tagpfx, eng=None):
                eng = eng or nc.gpsimd
                csc = pCS.tile([128, 512], f32, tag="cs", name=f"{tagpfx}c{ncI}")
                eng.dma_start(csc[:], cs[:, ts(ncI, 512)])
                snc = pCS.tile([128, 512], f32, tag="sn", name=f"{tagpfx}s{ncI}")
                eng.dma_start(snc[:], sn[:, ts(ncI, 512)])
                return csc[:], snc[:]

            def proj_rope_scatter(w_sb, xth, csc, snc, dst, ncI):
                # two RoPE'd maps -> scatter into dst[dims, head, tokens]
                for pair in (0, 1):
                    be, bo = 2 * pair, 2 * pair + 1
                    pe_ = pp.tile([128, 512], f32, tag="w")
                    po_ = pp.tile([128, 512], f32, tag="w")
                    for hf in range(2):
                        for ko in range(KO):
                            nc.tensor.matmul(pe_[:, ts(hf, 256)],
                                             w_sb[:, ko, ts(be, 128)],
                                             xth[hf][:, ko, :],
                                             start=(ko == 0), stop=(ko == KO - 1))
                        for ko in range(KO):
                            nc.tensor.matmul(po_[:, ts(hf, 256)],
                                             w_sb[:, ko, ts(bo, 128)],
                                             xth[hf][:, ko, :],
                                             start=(ko == 0), stop=(ko == KO - 1))
                    t1 = pT.tile([128, 512], f32, tag="t1")
                    t2 = pT.tile([128, 512], f32, tag="t2")
                    oe = pO.tile([128, 512], bf16, tag="oe")
                    oo = pO.tile([128, 512], bf16, tag="oo")
                    nc.vector.tensor_tensor(t1[:], pe_[:], csc, OP.mult)
                    nc.vector.tensor_tensor(t2[:], po_[:], snc, OP.mult)
                    nc.vector.tensor_tensor(oe[:], t1[:], t2[:], OP.subtract)
                    nc.vector.tensor_tensor(t1[:], pe_[:], snc, OP.mult)
                    nc.vector.tensor_tensor(t2[:], po_[:], csc, OP.mult)
                    nc.vector.tensor_tensor(oo[:], t1[:], t2[:], OP.add)
                    moff = pair * 64
                    for hh in range(HPC):
                        nc.gpsimd.dma_start(dst[ds(moff, 32), hh, ts(ncI, 512)],
                                            oe[ds(hh * 32, 32), :])
                        nc.gpsimd.dma_start(
                            dst[ds(moff + 32, 32), hh, ts(ncI, 512)],
                            oo[ds(hh * 32, 32), :])

            # ---- window 1: k + q projection into SBUF --------------------
            with tc.tile_pool(name="pW1", bufs=1) as pW1:
                wk_sb = pW1.tile([128, KO, 512], f32r)
                wq_sb = pW1.tile([128, KO, 512], f32r)
                for ko in range(KO):
                    nc.scalar.dma_start(wk_sb[:, ko, :], r(wk[ds(ko * 128, 128), :]))

                for ncI in range(NCH):
                    engs = (nc.gpsimd, nc.sync) if ncI == 0 else (nc.scalar, nc.sync)
                    xth = load_x(pW1, ncI, "k", engs)
                    if ncI == 0:
                        for ko in range(KO):
                            nc.sync.dma_start(wq_sb[:, ko, :],
                                              r(wq[ds(ko * 128, 128), :]))
                    csc, snc = load_cs(ncI, "k",
                                       nc.sync if ncI == 0 else nc.gpsimd)
                    proj_rope_scatter(wk_sb, xth, csc, snc, kt, ncI)
                    proj_rope_scatter(wq_sb, xth, csc, snc, qt, ncI)
                for ko in range(KO):
                    nc.sync.dma_start(wv_sb[:, ko, :], wv[ds(ko * 128, 128), :])

            # ---- window 2: v projection + attention + LN + @W_o ----------
            with tc.tile_pool(name="pW2", bufs=1) as pW2, \
                 tc.tile_pool(name="pA", bufs=2) as pA, \
                 tc.tile_pool(name="pE", bufs=3) as pE, \
                 tc.tile_pool(name="pU", bufs=3) as pU, \
                 tc.tile_pool(name="pST", bufs=4) as pST, \
                 tc.tile_pool(name="pES", bufs=2) as pES:
                wo_sb = pW2.tile([128, HPC, D], bf16)
                for hh in range(HPC):
                    nc.sync.dma_start(wo_sb[:, hh, :], wo[ds(hh * 128, 128), :])

                def emit_po(ncI, att_sb, qis=range(4)):
                    for qi in qis:
                        for nj in range(D // 512):
                            po = pp.tile([128, 512], f32, tag="w")
                            for h in range(HPC):
                                nc.tensor.matmul(
                                    po[:], att_sb[:, h, ts(qi, 128)],
                                    wo_sb[:, h, ts(nj, 512)],
                                    start=(h == 0), stop=(h == HPC - 1))
                            oo = pO.tile([128, 512], f32, tag="po", bufs=4)
                            nc.vector.tensor_copy(out=oo[:], in_=po[:])
                            nc.sync.dma_start(
                                out[ts(ncI * 4 + qi, 128), ts(nj, 512)], oo[:])

                def emit_v_part(ncI, xth, tsub, pv, kos):
                    for ko in kos:
                        nc.tensor.matmul(pv[:],
                                         xth[tsub // 2][:, ko,
                                                        ts(tsub % 2, 128)],
                                         wv_sb[:, ko, :], start=(ko == 0),
                                         stop=(ko == KO - 1))
                    if kos[-1] == KO - 1:
                        ov = pO.tile([128, 512], bf16, tag="ov")
                        nc.scalar.copy(out=ov[:], in_=pv[:])
                        nc.gpsimd.dma_start(vt[:, ncI * 4 + tsub, :, :], ov[:])

                def emit_v(ncI, xth, tsub):
                    pv = pp.tile([128, 512], f32, tag="w")
                    emit_v_part(ncI, xth, tsub, pv, list(range(KO)))

                prev_po = None
                xv_next = None
                for ncI in range(NCH):
                    if ncI == 0:
                        xth = load_xv(0)
                        for tsub in range(4):
                            emit_v(0, xth, tsub)
                    xv_next = load_xv(ncI + 1) if ncI + 1 < NCH else None

                    # -- attention row qc = ncI, all heads --
                    att_sb = pA.tile([128, HPC, 512], bf16, tag="att",
                                     name=f"att{ncI}")
                    var_all = pST.tile([128, HPC, 512], f32, tag="var", bufs=1,
                                       name=f"var{ncI}")
                    mu_all = pST.tile([128, HPC, 512], bf16, tag="mu", bufs=1,
                                      name=f"mu{ncI}")
                    klim = 4 * ncI + 4

                    for h in range(HPC):
                        def emit_scores(ki):
                            p = ki - 4 * ncI
                            c0 = 0 if p <= 0 else 128 * p
                            s12 = pp.tile([128, 2, 512], f32, tag="s12",
                                          bufs=2, name=f"s12_{ki}")
                            nc.tensor.matmul(s12[:, 0, c0:],
                                             kt[0:64, h, ts(ki, 128)],
                                             qt[0:64, h, ds(ncI * 512 + c0,
                                                            512 - c0)],
                                             start=True, stop=True,
                                             skip_group_check=True)
                            nc.tensor.matmul(s12[:, 1, c0:],
                                             kt[64:128, h, ts(ki, 128)],
                                             qt[64:128, h, ds(ncI * 512 + c0,
                                                              512 - c0)],
                                             start=True, stop=True,
                                             skip_group_check=True)
                            if p >= 0:
                                co = 128 * p
                                nc.vector.tensor_tensor(
                                    s12[:, :, co:co + 128],
                                    s12[:, :, co:co + 128], mw_sb[:, :, :],
                                    OP.add)
                            e12 = pE.tile([128, 2, 512], bf16, tag="e",
                                          name=f"e12_{ki}")
                            nc.scalar.activation(e12[:, :, c0:], s12[:, :, c0:],
                                                 AF.Exp, scale=SCALE)
                            return e12, c0

                        U1 = pp.tile([128, 512], f32, tag="w")
                        U2 = pp.tile([128, 512], f32, tag="w")
                        esum = pES.tile([128, 2, 512], f32r, tag="es")
                        # filler unit: prev chunk @W_o group, emitted near
                        # the end of the k-loop where the PE drains ahead of
                        # the exp pipeline
                        units = []
                        if prev_po is not None:
                            units.append(lambda h=h: emit_po(ncI - 1, prev_po,
                                                             qis=(h,)))
                        es = {0: emit_scores(0)}
                        for ki in range(klim):
                            if ki + 1 < klim:
                                es[ki + 1] = emit_scores(ki + 1)
                            e12, c0 = es.pop(ki)
                            st, sp = (ki == 0), (ki == klim - 1)
                            nc.tensor.matmul(U1[:, c0:], vt[:, ki, h, :],
                                             e12[:, 0, c0:], start=st, stop=sp)
                            nc.tensor.matmul(U2[:, c0:], vt[:, ki, h, :],
                                             e12[:, 1, c0:], start=st, stop=sp)
                            if ki == 0:
                                nc.gpsimd.tensor_copy(out=esum[:, :, :],
                                                      in_=e12[:, :, :])
                            else:
                                nc.gpsimd.tensor_tensor(
                                    esum[:, :, c0:], esum[:, :, c0:],
                                    e12[:, :, c0:], OP.add)
                        while units:
                            units.pop(0)()
                        # denominators from the Pool-accumulated exp sums
                        D1 = pp.tile([128, 512], f32, tag="w")
                        D2 = pp.tile([128, 512], f32, tag="w")
                        nc.tensor.matmul(D1[:], ones_cr[:], esum[:, 0, :],
                                         start=True, stop=True,
                                         skip_group_check=True)
                        nc.tensor.matmul(D2[:], ones_lr[:], esum[:, 1, :],
                                         start=True, stop=True,
                                         skip_group_check=True)
                        if xv_next is not None:
                            emit_v(ncI + 1, xv_next, h)
                        # epilogue: D replicated across partitions already
                        att = att_sb[:, h, :]
                        r1s = pU.tile([128, 512], f32, tag="u")
                        r2s = pU.tile([128, 512], f32, tag="u")
                        nc.vector.reciprocal(out=r1s[:], in_=D1[:])
                        nc.vector.tensor_tensor(r1s[:], U1[:], r1s[:], OP.mult)
                        nc.vector.reciprocal(out=r2s[:], in_=D2[:])
                        nc.vector.tensor_tensor(r2s[:], U2[:], r2s[:], OP.mult)
                        nc.vector.tensor_tensor(att, r1s[:], r2s[:], OP.subtract)
                        # LN stats (raw att): replicated ones-matmuls
                        sq = pU.tile([128, 512], bf16, tag="u")
                        nc.vector.tensor_tensor(sq[:], att, att, OP.mult)
                        MSm = pp.tile([128, 512], f32, tag="w")
                        MSs = pp.tile([128, 512], f32, tag="w")
                        nc.tensor.matmul(MSm[:], ones_m[:], att,
                                         start=True, stop=True,
                                         skip_group_check=True)
                        nc.tensor.matmul(MSs[:], ones_c[:], sq[:],
                                         start=True, stop=True,
                                         skip_group_check=True)
                        nc.vector.tensor_copy(out=mu_all[:, h, :], in_=MSm[:])
                        musq = pU.tile([128, 512], f32, tag="u")
                        nc.gpsimd.tensor_tensor(musq[:], mu_all[:, h, :],
                                                mu_all[:, h, :], OP.mult)
                        nc.vector.scalar_tensor_tensor(
                            var_all[:, h, :], MSs[:], 1.0 / DH, musq[:],
                            op0=OP.mult, op1=OP.subtract)

                    # batched LayerNorm: one eps-add + recip + Sqrt over all
                    # 4 heads (single ACT Sqrt per chunk -> 2 table loads)
                    nc.vector.tensor_scalar(
                        out=var_all[:, :, :], in0=var_all[:, :, :],
                        scalar1=1e-5, scalar2=None, op0=OP.add)
                    nc.vector.reciprocal(out=var_all[:, :, :],
                                         in_=var_all[:, :, :])
                    nc.scalar.activation(var_all[:, :, :], var_all[:, :, :],
                                         AF.Sqrt)
                    for h in range(HPC):
                        att = att_sb[:, h, :]
                        nc.gpsimd.tensor_tensor(mu_all[:, h, :],
                                                mu_all[:, h, :],
                                                var_all[:, h, :], OP.mult)
                        t1 = pU.tile([128, 512], f32, tag="u")
                        nc.gpsimd.tensor_tensor(t1[:], att, var_all[:, h, :],
                                                OP.mult)
                        nc.gpsimd.tensor_tensor(t1[:], t1[:], mu_all[:, h, :],
                                                OP.subtract)
                        nc.gpsimd.tensor_scalar(
                            out=att, in0=t1[:],
                            scalar1=gb_sb[:, h:h + 1],
                            scalar2=gb_sb[:, HPC + h:HPC + h + 1],
                            op0=OP.mult, op1=OP.add)
                    prev_po = att_sb
                emit_po(NCH - 1, prev_po)

    nc.finalize()
    return nc


def get_program(s=S):
    if s not in _PROGRAM_CACHE:
        _PROGRAM_CACHE[s] = build_program(s)
    return _PROGRAM_CACHE[s]


def make_core_inputs(x, cos, sin, W_qkv, W_o, ln_gamma, ln_beta, lbda, core, s=S):
    """Host-side shard prep for one core."""
    b, hg = core // 4, core % 4
    heads = list(range(hg * HPC, (hg + 1) * HPC))

    def qk_block_cols(base, dstart):
        # even/odd pair columns for one 32-wide block across the 4 heads
        return [base + hh * DH + dstart + 2 * p for hh in heads for p in range(32)]

    def section(base):
        cols = []
        for dstart in (0, 1, HD, HD + 1):     # m1-even, m1-odd, m2-even, m2-odd
            cols += qk_block_cols(base, dstart)
        return cols

    import ml_dtypes
    bf = ml_dtypes.bfloat16
    wq_ = np.ascontiguousarray(W_qkv[:, section(0)], dtype=np.float32)
    wk_ = np.ascontiguousarray(W_qkv[:, section(D)], dtype=np.float32)
    vcols = [2 * D + hh * DH + dd for hh in heads for dd in range(DH)]
    wv = np.ascontiguousarray(W_qkv[:, vcols].astype(bf))
    worows = [hh * DH + dd for hh in heads for dd in range(DH)]
    wo = np.ascontiguousarray(W_o[worows, :].astype(bf))

    xT = np.ascontiguousarray(x[b].T, dtype=np.float32)
    xTb = np.ascontiguousarray(xT.astype(bf))
    cst = np.ascontiguousarray(np.tile(cos.T, (HPC, 1)), dtype=np.float32)
    snt = np.ascontiguousarray(np.tile(sin.T, (HPC, 1)), dtype=np.float32)

    gb = np.zeros((128, 2 * HPC), dtype=np.float32)
    for j, hh in enumerate(heads):
        gb[:, j] = ln_gamma[hh] * OUT_MULT
        gb[:, HPC + j] = ln_beta[hh] * OUT_MULT

    diag = np.where(np.triu(np.ones((128, 128), dtype=bool)), 0.0, NEG)
    mwide = np.ascontiguousarray(
        np.concatenate([diag, diag], axis=1).astype(np.float32))

    return {
        "xT": xT, "xTb": xTb, "wk": wk_, "wq": wq_, "wv": wv, "wo": wo,
        "cs": cst, "sn": snt, "gb": gb, "mw": mwide,
        "onem": np.full((128, 128), 1.0 / DH, dtype=bf),
        "onec": np.ones((128, 128), dtype=bf),
        "onel": np.full((128, 128), 1.0 / lbda if lbda != 0 else 1e30,
                        dtype=bf),
        "onecr": np.ones((128, 128), dtype=np.float32),
        "onelr": np.full((128, 128), 1.0 / lbda if lbda != 0 else 1e30,
                         dtype=np.float32),
    }


def _mask_is_causal(mask, s=S):
    m = np.asarray(mask).reshape(s, s)
    tril = np.tril(np.ones((s, s), dtype=bool))
    if not np.array_equal(m == 0.0, tril):
        return False
    off = m[~tril]
    return off.size == 0 or (np.all(off <= -1.0e8) and np.all(np.isfinite(off)))


def _numpy_reference(x, mask, cos, sin, W_qkv, W_o, ln_gamma, ln_beta, lbda):
    """Exact-math fallback (used only if the mask is not the causal pattern)."""
    b, s, d = x.shape
    qkv = x @ W_qkv
    q, k, v = np.split(qkv, 3, axis=-1)
    q = q.reshape(b, s, H, DH).transpose(0, 2, 1, 3)
    k = k.reshape(b, s, H, DH).transpose(0, 2, 1, 3)
    v = v.reshape(b, s, H, DH).transpose(0, 2, 1, 3)

    def rope(t):
        tr = t.reshape(b, H, s, HD // 2, 2)
        x1, x2 = tr[..., 0], tr[..., 1]
        c = cos[None, None]
        sn_ = sin[None, None]
        o1 = x1 * c - x2 * sn_
        o2 = x1 * sn_ + x2 * c
        return np.stack([o1, o2], axis=-1).reshape(b, H, s, HD)

    q1, q2 = q[..., :HD], q[..., HD:]
    k1, k2 = k[..., :HD], k[..., HD:]
    q1, k1 = rope(q1), rope(k1)
    q2, k2 = rope(q2), rope(k2)

    def softm(z):
        z = z - z.max(-1, keepdims=True)
        e = np.exp(z)
        return e / e.sum(-1, keepdims=True)

    m = np.asarray(mask).reshape(1, 1, s, s)
    a1 = softm(np.einsum("bhqd,bhkd->bhqk", q1, k1) * SCALE + m)
    a2 = softm(np.einsum("bhqd,bhkd->bhqk", q2, k2) * SCALE + m)
    a = a1 - float(lbda) * a2
    o = np.einsum("bhqk,bhkd->bhqd", a, v)
    mu = o.mean(-1, keepdims=True)
    var = o.var(-1, keepdims=True)
    o = (o - mu) / np.sqrt(var + 1e-5)
    o = o * ln_gamma[None, :, None, :] + ln_beta[None, :, None, :]
    o = o * OUT_MULT
    o = o.transpose(0, 2, 1, 3).reshape(b, s, d)
    return (o @ W_o).astype(np.float32)


def kernel(x, mask, cos, sin, W_qkv, W_o, ln_gamma, ln_beta, lbda):
    global LAST_RESULTS, LAST_EXEC_NS
    x = np.asarray(x, dtype=np.float32)
    cos = np.asarray(cos, dtype=np.float32)
    sin = np.asarray(sin, dtype=np.float32)
    W_qkv = np.asarray(W_qkv, dtype=np.float32)
    W_o = np.asarray(W_o, dtype=np.float32)
    ln_gamma = np.asarray(ln_gamma, dtype=np.float32)
    ln_beta = np.asarray(ln_beta, dtype=np.float32)
    lbda_f = float(np.asarray(lbda))

    if not _mask_is_causal(mask):
        return _numpy_reference(x, mask, cos, sin, W_qkv, W_o,
                                ln_gamma, ln_beta, lbda_f)

    from concourse.bass_utils import run_bass_kernel_spmd

    nc = get_program(S)
    in_maps = [
        make_core_inputs(x, cos, sin, W_qkv, W_o, ln_gamma, ln_beta, lbda_f, c)
        for c in range(NCORES)
    ]
    kwargs = {"trace": TRACE}
    if TRACE and TRACE_DIR:
        kwargs["tmpdir"] = TRACE_DIR
    res = run_bass_kernel_spmd(nc, in_maps, core_ids=list(range(NCORES)),
                               **kwargs)
    LAST_RESULTS = res
    LAST_EXEC_NS = getattr(res, "exec_time_ns", None)

    outf = np.zeros((B, S, D), dtype=np.float32)
    for c in range(NCORES):
        outf[c // 4] += res.results[c]["out"]
    return outf


# revision 42
# speedup vs baseline: 1.1904x; 1.0246x over previous
"""Differential attention (B=2, S=2048, D=2048, H=16) on 8 Trainium2 cores.

Sharding: core c -> batch b=c//4, head group hg=c%4 (4 heads each).
Each core computes qkv projection for its head columns, RoPE, differential
attention, per-head LayerNorm, and a partial @W_o over its 512 vd rows.
Host sums the 4 partials per batch. No duplicated FLOPs, no collectives.

Pipeline (single pass, no DRAM scratch):
 - window1: per 512-token chunk, project k (2 maps, RoPE'd) and v, scatter
   straight into SBUF-resident kt/vt tiles via SBUF->SBUF DMA.
 - window2: per chunk, project q (RoPE'd, SBUF-resident), then immediately
   run the attention row qc=chunk for all 4 heads (scores transposed [k,q],
   fused two-map PSUM tile + single exp per k-tile, fp32r matmuls),
   LayerNorm stats via replicated ones-matmuls (no broadcast DMAs), and the
   partial @W_o for that chunk row. PE never waits on a phase barrier.
 - diagonal k-tiles padded to >=256 moving columns (fp32r runs 4x slower
   below 256) with a NEG|diag mask block.
"""
import sys

sys.path.insert(0, "/opt/trn_rl_repo")

import numpy as np

B, S, D = 2, 2048, 2048
H = 16
HD = D // (2 * H)          # 64 per-map head dim
DH = 2 * HD                # 128 per-head dim
HPC = H // 4               # 4 heads per core
NCORES = 8
SCALE = HD ** -0.5         # 0.125
NEG = -8.0e9               # mask add value pre-scale (-1e9 / SCALE)
OUT_MULT = 1.0 - 0.8       # (1 - LBDA_INIT)

TRACE = False
TRACE_DIR = None
LAST_RESULTS = None
LAST_EXEC_NS = None

_PROGRAM_CACHE = {}


def build_program(s=S):
    """Build the per-core Bass program (SPMD: same program, 8 cores)."""
    import concourse.bass as bass
    import concourse.tile as tile
    from concourse import bacc, mybir
    from concourse.bass import ts, ds

    f32 = mybir.dt.float32
    f32r = mybir.dt.float32r
    bf16 = mybir.dt.bfloat16
    AF = mybir.ActivationFunctionType
    OP = mybir.AluOpType

    NCH = s // 512              # token chunks of 512
    KT = s // 128               # k tiles of 128
    KO = D // 128               # contraction chunks over D

    nc = bacc.Bacc()
    xT = nc.declare_dram_parameter("xT", [D, s], f32, isOutput=False)
    xTb = nc.declare_dram_parameter("xTb", [D, s], bf16, isOutput=False)
    wk = nc.declare_dram_parameter("wk", [D, 4 * 128], f32, isOutput=False)
    wq = nc.declare_dram_parameter("wq", [D, 4 * 128], f32, isOutput=False)
    wv = nc.declare_dram_parameter("wv", [D, HPC * DH], bf16, isOutput=False)
    wo = nc.declare_dram_parameter("wo", [HPC * DH, D], bf16, isOutput=False)
    cs = nc.declare_dram_parameter("cs", [128, s], f32, isOutput=False)
    sn = nc.declare_dram_parameter("sn", [128, s], f32, isOutput=False)
    gb = nc.declare_dram_parameter("gb", [128, 2 * HPC], f32, isOutput=False)
    mw = nc.declare_dram_parameter("mw", [128, 256], f32, isOutput=False)
    onem = nc.declare_dram_parameter("onem", [128, 128], bf16, isOutput=False)
    onec = nc.declare_dram_parameter("onec", [128, 128], bf16, isOutput=False)
    onel = nc.declare_dram_parameter("onel", [128, 128], bf16, isOutput=False)
    onecr = nc.declare_dram_parameter("onecr", [128, 128], f32, isOutput=False)
    onelr = nc.declare_dram_parameter("onelr", [128, 128], f32, isOutput=False)
    out = nc.declare_dram_parameter("out", [s, D], f32, isOutput=True)

    r = lambda ap: ap.bitcast(f32r)

    with tile.TileContext(nc) as tc:
        with tc.tile_pool(name="pL", bufs=1) as pL, \
             tc.tile_pool(name="pO", bufs=2) as pO, \
             tc.tile_pool(name="pT", bufs=2) as pT, \
             tc.tile_pool(name="pCS", bufs=2) as pCS, \
             tc.tile_pool(name="pp", bufs=4, space="PSUM") as pp:
            # ---- long-lived SBUF state -----------------------------------
            kt = pL.tile([128, HPC, s], bf16)       # per-head k dims x tokens
            qt = pL.tile([128, HPC, s], bf16)       # per-head q dims x tokens
            vt = pL.tile([128, KT, HPC, DH], bf16)  # k-token x (ki, h, vd)
            wv_sb = pL.tile([128, KO, 512], bf16)
            gb_sb = pL.tile([128, 2 * HPC], f32)
            nc.sync.dma_start(gb_sb[:], gb[:])
            mw_sb = pL.tile([128, 2, 128], f32)
            nc.sync.dma_start(mw_sb[:, :, :], mw[:])
            ones_m = pL.tile([128, 128], bf16)
            nc.sync.dma_start(ones_m[:], onem[:])
            ones_c = pL.tile([128, 128], bf16)
            nc.sync.dma_start(ones_c[:], onec[:])
            ones_l = pL.tile([128, 128], bf16)
            nc.sync.dma_start(ones_l[:], onel[:])
            ones_cr = pL.tile([128, 128], f32r)
            nc.sync.dma_start(ones_cr[:], r(onecr[:]))
            ones_lr = pL.tile([128, 128], f32r)
            nc.sync.dma_start(ones_lr[:], r(onelr[:]))

            def load_x(pool, ncI, tagpfx, engs):
                xth = []
                for hf in range(2):
                    eng = engs[hf]
                    xh = pool.tile([128, KO, 256], f32r, tag="xt", bufs=2,
                                   name=f"x{tagpfx}{ncI}_{hf}")
                    for ko in range(KO):
                        eng.dma_start(
                            xh[:, ko, :],
                            r(xT[ds(ko * 128, 128),
                                 ds(ncI * 512 + hf * 256, 256)]))
                    xth.append(xh)
                return xth

            def load_xv(ncI, engs=(None, None)):
                xth = []
                for hf in range(2):
                    eng = engs[hf] or nc.sync
                    xh = pW2.tile([128, KO, 256], bf16, tag="xv", bufs=2,
                                  name=f"xv{ncI}_{hf}")
                    for ko in range(KO):
                        eng.dma_start(
                            xh[:, ko, :],
                            xTb[ds(ko * 128, 128),
                                ds(ncI * 512 + hf * 256, 256)])
                    xth.append(xh)
                return xth

            def load_cs(ncI, tagpfx, eng=None):
                eng = eng or nc.gpsimd
                csc = pCS.tile([128, 512], f32, tag="cs", name=f"{tagpfx}c{ncI}")
                eng.dma_start(csc[:], cs[:, ts(ncI, 512)])
                snc = pCS.tile([128, 512], f32, tag="sn", name=f"{tagpfx}s{ncI}")
                eng.dma_start(snc[:], sn[:, ts(ncI, 512)])
                return csc[:], snc[:]

            def proj_rope_scatter(w_sb, xth, csc, snc, dst, ncI):
                # two RoPE'd maps -> scatter into dst[dims, head, tokens]
                for pair in (0, 1):
                    be, bo = 2 * pair, 2 * pair + 1
                    pe_ = pp.tile([128, 512], f32, tag="w")
                    po_ = pp.tile([128, 512], f32, tag="w")
                    for hf in range(2):
                        for ko in range(KO):
                            nc.tensor.matmul(pe_[:, ts(hf, 256)],
                                             w_sb[:, ko, ts(be, 128)],
                                             xth[hf][:, ko, :],
                                             start=(ko == 0), stop=(ko == KO - 1))
                        for ko in range(KO):
                            nc.tensor.matmul(po_[:, ts(hf, 256)],
                                             w_sb[:, ko, ts(bo, 128)],
                                             xth[hf][:, ko, :],
                                             start=(ko == 0), stop=(ko == KO - 1))
                    t1 = pT.tile([128, 512], f32, tag="t1")
                    t2 = pT.tile([128, 512], f32, tag="t2")
                    oe = pO.tile([128, 512], bf16, tag="oe")
                    oo = pO.tile([128, 512], bf16, tag="oo")
                    nc.vector.tensor_tensor(t1[:], pe_[:], csc, OP.mult)
                    nc.vector.tensor_tensor(t2[:], po_[:], snc, OP.mult)
                    nc.vector.tensor_tensor(oe[:], t1[:], t2[:], OP.subtract)
                    nc.vector.tensor_tensor(t1[:], pe_[:], snc, OP.mult)
                    nc.vector.tensor_tensor(t2[:], po_[:], csc, OP.mult)
                    nc.vector.tensor_tensor(oo[:], t1[:], t2[:], OP.add)
                    moff = pair * 64
                    for hh in range(HPC):
                        nc.gpsimd.dma_start(dst[ds(moff, 32), hh, ts(ncI, 512)],
                                            oe[ds(hh * 32, 32), :])
                        nc.gpsimd.dma_start(
                            dst[ds(moff + 32, 32), hh, ts(ncI, 512)],
                            oo[ds(hh * 32, 32), :])

            # ---- window 1: k + q projection into SBUF --------------------
            with tc.tile_pool(name="pW1", bufs=1) as pW1:
                wk_sb = pW1.tile([128, KO, 512], f32r)
                wq_sb = pW1.tile([128, KO, 512], f32r)
                for ko in range(KO):
                    nc.scalar.dma_start(wk_sb[:, ko, :], r(wk[ds(ko * 128, 128), :]))

                for ncI in range(NCH):
                    engs = (nc.gpsimd, nc.sync) if ncI == 0 else (nc.scalar, nc.sync)
                    xth = load_x(pW1, ncI, "k", engs)
                    if ncI == 0:
                        for ko in range(KO):
                            nc.sync.dma_start(wq_sb[:, ko, :],
                                              r(wq[ds(ko * 128, 128), :]))
                    csc, snc = load_cs(ncI, "k",
                                       nc.sync if ncI == 0 else nc.gpsimd)
                    proj_rope_scatter(wk_sb, xth, csc, snc, kt, ncI)
                    if ncI == 0:
                        # warm the Exp table while ACT is otherwise idle
                        warm = pL.tile([1, 4], f32)
                        nc.vector.memset(warm[:], 0.0)
                        nc.scalar.activation(warm[:], warm[:], AF.Exp)
                    proj_rope_scatter(wq_sb, xth, csc, snc, qt, ncI)
                for ko in range(KO):
                    nc.sync.dma_start(wv_sb[:, ko, :], wv[ds(ko * 128, 128), :])

            # ---- window 2: v projection + attention + LN + @W_o ----------
            with tc.tile_pool(name="pW2", bufs=1) as pW2, \
                 tc.tile_pool(name="pA", bufs=3) as pA, \
                 tc.tile_pool(name="pE", bufs=3) as pE, \
                 tc.tile_pool(name="pU", bufs=3) as pU, \
                 tc.tile_pool(name="pST", bufs=4) as pST, \
                 tc.tile_pool(name="pES", bufs=1) as pES:
                wo_sb = pW2.tile([128, HPC, D], bf16)
                for hh in range(HPC):
                    nc.sync.dma_start(wo_sb[:, hh, :], wo[ds(hh * 128, 128), :])

                def emit_po(ncI, att_sb, qis=range(4)):
                    for qi in qis:
                        for nj in range(D // 512):
                            po = pp.tile([128, 512], f32, tag="w")
                            for h in range(HPC):
                                nc.tensor.matmul(
                                    po[:], att_sb[:, h, ts(qi, 128)],
                                    wo_sb[:, h, ts(nj, 512)],
                                    start=(h == 0), stop=(h == HPC - 1))
                            oo = pO.tile([128, 512], f32, tag="po", bufs=4)
                            nc.vector.tensor_copy(out=oo[:], in_=po[:])
                            nc.sync.dma_start(
                                out[ts(ncI * 4 + qi, 128), ts(nj, 512)], oo[:])

                def emit_v_part(ncI, xth, tsub, pv, kos):
                    for ko in kos:
                        nc.tensor.matmul(pv[:],
                                         xth[tsub // 2][:, ko,
                                                        ts(tsub % 2, 128)],
                                         wv_sb[:, ko, :], start=(ko == 0),
                                         stop=(ko == KO - 1))
                    if kos[-1] == KO - 1:
                        ov = pO.tile([128, 512], bf16, tag="ov")
                        nc.vector.tensor_copy(out=ov[:], in_=pv[:])
                        nc.gpsimd.dma_start(vt[:, ncI * 4 + tsub, :, :], ov[:])

                def emit_v(ncI, xth, tsub):
                    pv = pp.tile([128, 512], f32, tag="w")
                    emit_v_part(ncI, xth, tsub, pv, list(range(KO)))

                prev_po = None
                prev_prev_po = None
                xv_next = None
                for ncI in range(NCH):
                    if ncI == 0:
                        xth = load_xv(0)
                        for tsub in range(4):
                            emit_v(0, xth, tsub)
                    xv_next = load_xv(ncI + 1) if ncI + 1 < NCH else None

                    # -- attention row qc = ncI, all heads --
                    att_sb = pA.tile([128, HPC, 512], bf16, tag="att",
                                     name=f"att{ncI}")
                    var_all = pST.tile([128, HPC, 512], f32, tag="var", bufs=1,
                                       name=f"var{ncI}")
                    mu_all = pST.tile([128, HPC, 512], bf16, tag="mu", bufs=1,
                                      name=f"mu{ncI}")
                    klim = 4 * ncI + 4

                    for h in range(HPC):
                        def emit_scores(ki):
                            p = ki - 4 * ncI
                            c0 = 0 if p <= 0 else 128 * p
                            s12 = pp.tile([128, 2, 512], f32, tag="s12",
                                          bufs=2, name=f"s12_{ki}")
                            nc.tensor.matmul(s12[:, 0, c0:],
                                             kt[0:64, h, ts(ki, 128)],
                                             qt[0:64, h, ds(ncI * 512 + c0,
                                                            512 - c0)],
                                             start=True, stop=True,
                                             skip_group_check=True)
                            nc.tensor.matmul(s12[:, 1, c0:],
                                             kt[64:128, h, ts(ki, 128)],
                                             qt[64:128, h, ds(ncI * 512 + c0,
                                                              512 - c0)],
                                             start=True, stop=True,
                                             skip_group_check=True)
                            if p >= 0:
                                co = 128 * p
                                nc.vector.tensor_tensor(
                                    s12[:, :, co:co + 128],
                                    s12[:, :, co:co + 128], mw_sb[:, :, :],
                                    OP.add)
                            e12 = pE.tile([128, 2, 512], bf16, tag="e", bufs=4,
                                          name=f"e12_{ki}")
                            nc.scalar.activation(e12[:, :, c0:], s12[:, :, c0:],
                                                 AF.Exp, scale=SCALE)
                            return e12, c0

                        U1 = pp.tile([128, 512], f32, tag="w")
                        U2 = pp.tile([128, 512], f32, tag="w")
                        esum = pES.tile([128, 2, 512], f32r, tag="es")
                        # filler unit: prev chunk @W_o group, emitted near
                        # the end of the k-loop where the PE drains ahead of
                        # the exp pipeline
                        units = []
                        if h == 0 and prev_prev_po is not None:
                            units.append(
                                lambda: emit_po(ncI - 2, prev_prev_po,
                                                qis=(0,)))
                        elif h > 0 and prev_po is not None:
                            units.append(lambda h=h: emit_po(ncI - 1, prev_po,
                                                             qis=(h,)))
                        es = {0: emit_scores(0)}
                        for ki in range(klim):
                            if ki + 1 < klim:
                                es[ki + 1] = emit_scores(ki + 1)
                            e12, c0 = es.pop(ki)
                            st, sp = (ki == 0), (ki == klim - 1)
                            nc.tensor.matmul(U1[:, c0:], vt[:, ki, h, :],
                                             e12[:, 0, c0:], start=st, stop=sp)
                            nc.tensor.matmul(U2[:, c0:], vt[:, ki, h, :],
                                             e12[:, 1, c0:], start=st, stop=sp)
                            if ki == 0:
                                nc.gpsimd.tensor_copy(out=esum[:, :, :],
                                                      in_=e12[:, :, :])
                            else:
                                nc.gpsimd.tensor_tensor(
                                    esum[:, :, c0:], esum[:, :, c0:],
                                    e12[:, :, c0:], OP.add)
                        while units:
                            units.pop(0)()
                        # denominators from the Pool-accumulated exp sums
                        D1 = pp.tile([128, 512], f32, tag="w")
                        D2 = pp.tile([128, 512], f32, tag="w")
                        nc.tensor.matmul(D1[:], ones_cr[:], esum[:, 0, :],
                                         start=True, stop=True,
                                         skip_group_check=True)
                        nc.tensor.matmul(D2[:], ones_lr[:], esum[:, 1, :],
                                         start=True, stop=True,
                                         skip_group_check=True)
                        if xv_next is not None:
                            emit_v(ncI + 1, xv_next, h)
                        # epilogue: D replicated across partitions already
                        att = att_sb[:, h, :]
                        r1s = pU.tile([128, 512], f32, tag="u")
                        r2s = pU.tile([128, 512], f32, tag="u")
                        nc.vector.reciprocal(out=r1s[:], in_=D1[:])
                        nc.vector.tensor_tensor(r1s[:], U1[:], r1s[:], OP.mult)
                        nc.vector.reciprocal(out=r2s[:], in_=D2[:])
                        nc.vector.tensor_tensor(r2s[:], U2[:], r2s[:], OP.mult)
                        nc.vector.tensor_tensor(att, r1s[:], r2s[:], OP.subtract)
                        # LN stats (raw att): replicated ones-matmuls
                        sq = pU.tile([128, 512], bf16, tag="u")
                        nc.vector.tensor_tensor(sq[:], att, att, OP.mult)
                        MSm = pp.tile([128, 512], f32, tag="w")
                        MSs = pp.tile([128, 512], f32, tag="w")
                        nc.tensor.matmul(MSm[:], ones_m[:], att,
                                         start=True, stop=True,
                                         skip_group_check=True)
                        nc.tensor.matmul(MSs[:], ones_c[:], sq[:],
                                         start=True, stop=True,
                                         skip_group_check=True)
                        nc.vector.tensor_copy(out=mu_all[:, h, :], in_=MSm[:])
                        musq = pU.tile([128, 512], f32, tag="u")
                        nc.gpsimd.tensor_tensor(musq[:], mu_all[:, h, :],
                                                mu_all[:, h, :], OP.mult)
                        nc.vector.scalar_tensor_tensor(
                            var_all[:, h, :], MSs[:], 1.0 / DH, musq[:],
                            op0=OP.mult, op1=OP.subtract)

                    # batched LayerNorm: one eps-add + recip + Sqrt over all
                    # 4 heads (single ACT Sqrt per chunk -> 2 table loads)
                    nc.vector.tensor_scalar(
                        out=var_all[:, :, :], in0=var_all[:, :, :],
                        scalar1=1e-5, scalar2=None, op0=OP.add)
                    nc.vector.reciprocal(out=var_all[:, :, :],
                                         in_=var_all[:, :, :])
                    nc.scalar.activation(var_all[:, :, :], var_all[:, :, :],
                                         AF.Sqrt)
                    for h in range(HPC):
                        att = att_sb[:, h, :]
                        nc.gpsimd.tensor_tensor(mu_all[:, h, :],
                                                mu_all[:, h, :],
                                                var_all[:, h, :], OP.mult)
                        t1 = pU.tile([128, 512], f32, tag="u")
                        nc.gpsimd.tensor_tensor(t1[:], att, var_all[:, h, :],
                                                OP.mult)
                        nc.gpsimd.tensor_tensor(t1[:], t1[:], mu_all[:, h, :],
                                                OP.subtract)
                        nc.gpsimd.tensor_scalar(
                            out=att, in0=t1[:],
                            scalar1=gb_sb[:, h:h + 1],
                            scalar2=gb_sb[:, HPC + h:HPC + h + 1],
                            op0=OP.mult, op1=OP.add)
                    prev_prev_po = prev_po
                    prev_po = att_sb
                emit_po(NCH - 2, prev_prev_po, qis=(0,))
                emit_po(NCH - 1, prev_po)

    nc.finalize()
    return nc


def get_program(s=S):
    if s not in _PROGRAM_CACHE:
        _PROGRAM_CACHE[s] = build_program(s)
    return _PROGRAM_CACHE[s]


def make_core_inputs(x, cos, sin, W_qkv, W_o, ln_gamma, ln_beta, lbda, core, s=S):
    """Host-side shard prep for one core."""
    b, hg = core // 4, core % 4
    heads = list(range(hg * HPC, (hg + 1) * HPC))

    def qk_block_cols(base, dstart):
        # even/odd pair columns for one 32-wide block across the 4 heads
        return [base + hh * DH + dstart + 2 * p for hh in heads for p in range(32)]

    def section(base):
        cols = []
        for dstart in (0, 1, HD, HD + 1):     # m1-even, m1-odd, m2-even, m2-odd
            cols += qk_block_cols(base, dstart)
        return cols

    import ml_dtypes
    bf = ml_dtypes.bfloat16
    wq_ = np.ascontiguousarray(W_qkv[:, section(0)], dtype=np.float32)
    wk_ = np.ascontiguousarray(W_qkv[:, section(D)], dtype=np.float32)
    vcols = [2 * D + hh * DH + dd for hh in heads for dd in range(DH)]
    wv = np.ascontiguousarray(W_qkv[:, vcols].astype(bf))
    worows = [hh * DH + dd for hh in heads for dd in range(DH)]
    wo = np.ascontiguousarray(W_o[worows, :].astype(bf))

    xT = np.ascontiguousarray(x[b].T, dtype=np.float32)
    xTb = np.ascontiguousarray(xT.astype(bf))
    cst = np.ascontiguousarray(np.tile(cos.T, (HPC, 1)), dtype=np.float32)
    snt = np.ascontiguousarray(np.tile(sin.T, (HPC, 1)), dtype=np.float32)

    gb = np.zeros((128, 2 * HPC), dtype=np.float32)
    for j, hh in enumerate(heads):
        gb[:, j] = ln_gamma[hh] * OUT_MULT
        gb[:, HPC + j] = ln_beta[hh] * OUT_MULT

    diag = np.where(np.triu(np.ones((128, 128), dtype=bool)), 0.0, NEG)
    mwide = np.ascontiguousarray(
        np.concatenate([diag, diag], axis=1).astype(np.float32))

    return {
        "xT": xT, "xTb": xTb, "wk": wk_, "wq": wq_, "wv": wv, "wo": wo,
        "cs": cst, "sn": snt, "gb": gb, "mw": mwide,
        "onem": np.full((128, 128), 1.0 / DH, dtype=bf),
        "onec": np.ones((128, 128), dtype=bf),
        "onel": np.full((128, 128), 1.0 / lbda if lbda != 0 else 1e30,
                        dtype=bf),
        "onecr": np.ones((128, 128), dtype=np.float32),
        "onelr": np.full((128, 128), 1.0 / lbda if lbda != 0 else 1e30,
                         dtype=np.float32),
    }


def _mask_is_causal(mask, s=S):
    m = np.asarray(mask).reshape(s, s)
    tril = np.tril(np.ones((s, s), dtype=bool))
    if not np.array_equal(m == 0.0, tril):
        return False
    off = m[~tril]
    return off.size == 0 or (np.all(off <= -1.0e8) and np.all(np.isfinite(off)))


def _numpy_reference(x, mask, cos, sin, W_qkv, W_o, ln_gamma, ln_beta, lbda):
    """Exact-math fallback (used only if the mask is not the causal pattern)."""
    b, s, d = x.shape
    qkv = x @ W_qkv
    q, k, v = np.split(qkv, 3, axis=-1)
    q = q.reshape(b, s, H, DH).transpose(0, 2, 1, 3)
    k = k.reshape(b, s, H, DH).transpose(0, 2, 1, 3)
    v = v.reshape(b, s, H, DH).transpose(0, 2, 1, 3)

    def rope(t):
        tr = t.reshape(b, H, s, HD // 2, 2)
        x1, x2 = tr[..., 0], tr[..., 1]
        c = cos[None, None]
        sn_ = sin[None, None]
        o1 = x1 * c - x2 * sn_
        o2 = x1 * sn_ + x2 * c
        return np.stack([o1, o2], axis=-1).reshape(b, H, s, HD)

    q1, q2 = q[..., :HD], q[..., HD:]
    k1, k2 = k[..., :HD], k[..., HD:]
    q1, k1 = rope(q1), rope(k1)
    q2, k2 = rope(q2), rope(k2)

    def softm(z):
        z = z - z.max(-1, keepdims=True)
        e = np.exp(z)
        return e / e.sum(-1, keepdims=True)

    m = np.asarray(mask).reshape(1, 1, s, s)
    a1 = softm(np.einsum("bhqd,bhkd->bhqk", q1, k1) * SCALE + m)
    a2 = softm(np.einsum("bhqd,bhkd->bhqk", q2, k2) * SCALE + m)
    a = a1 - float(lbda) * a2
    o = np.einsum("bhqk,bhkd->bhqd", a, v)
    mu = o.mean(-1, keepdims=True)
    var = o.var(-1, keepdims=True)
    o = (o - mu) / np.sqrt(var + 1e-5)
    o = o * ln_gamma[None, :, None, :] + ln_beta[None, :, None, :]
    o = o * OUT_MULT
    o = o.transpose(0, 2, 1, 3).reshape(b, s, d)
    return (o @ W_o).astype(np.float32)


def kernel(x, mask, cos, sin, W_qkv, W_o, ln_gamma, ln_beta, lbda):
    global LAST_RESULTS, LAST_EXEC_NS
    x = np.asarray(x, dtype=np.float32)
    cos = np.asarray(cos, dtype=np.float32)
    sin = np.asarray(sin, dtype=np.float32)
    W_qkv = np.asarray(W_qkv, dtype=np.float32)
    W_o = np.asarray(W_o, dtype=np.float32)
    ln_gamma = np.asarray(ln_gamma, dtype=np.float32)
    ln_beta = np.asarray(ln_beta, dtype=np.float32)
    lbda_f = float(np.asarray(lbda))

    if not _mask_is_causal(mask):
        return _numpy_reference(x, mask, cos, sin, W_qkv, W_o,
                                ln_gamma, ln_beta, lbda_f)

    from concourse.bass_utils import run_bass_kernel_spmd

    nc = get_program(S)
    in_maps = [
        make_core_inputs(x, cos, sin, W_qkv, W_o, ln_gamma, ln_beta, lbda_f, c)
        for c in range(NCORES)
    ]
    kwargs = {"trace": TRACE}
    if TRACE and TRACE_DIR:
        kwargs["tmpdir"] = TRACE_DIR
    res = run_bass_kernel_spmd(nc, in_maps, core_ids=list(range(NCORES)),
                               **kwargs)
    LAST_RESULTS = res
    LAST_EXEC_NS = getattr(res, "exec_time_ns", None)

    outf = np.zeros((B, S, D), dtype=np.float32)
    for c in range(NCORES):
        outf[c // 4] += res.results[c]["out"]
    return outf


# revision 47
# speedup vs baseline: 1.2107x; 1.0171x over previous
"""Differential attention (B=2, S=2048, D=2048, H=16) on 8 Trainium2 cores.

Sharding: core c -> batch b=c//4, head group hg=c%4 (4 heads each).
Each core computes the qkv projection for its head columns, RoPE, differential
attention, per-head LayerNorm, and a partial @W_o over its 512 vd rows.
Host sums the 4 partials per batch. No duplicated FLOPs, no collectives.

Single-pass pipeline, no DRAM scratch:
 - window1: per 512-token chunk, project k and q (fp32r x/weights), RoPE on
   DVE, scatter bf16 results straight into SBUF-resident kt/qt via
   SBUF->SBUF DMA.  Loads are spread over the ACT/SP/Pool DMA queues.
 - window2: per chunk, project v (bf16 path), then run attention row qc =
   chunk for all 4 heads: scores transposed [k,q] (bf16 kt x qt), fused
   two-map PSUM score tile + one exp per k-tile (bf16 e12), U = v.T e on PE,
   exp-sums accumulated on the Pool engine (f32r esum) so the softmax
   denominators cost only 2 small PE matmuls per head, LayerNorm stats via
   replicated ones-matmuls (no broadcast DMAs), one batched Sqrt per chunk
   (2 ACT table loads per chunk), and the partial @W_o emitted as per-head
   filler groups one chunk behind so the PE rides out exp/norm latency.
 - PSUM: 4 banks score lookahead (2 x [128,2,512]) + 4 banks shared among
   qkv-projection groups, U1/U2, D1/D2, LN-stats and @W_o accumulators.

bf16 is used where the 2e-2 tolerance allows (v path, k/q storage, e12,
attn output, W_o); measured rel err ~5e-3.  GPSIMD never touches PSUM and
no instruction reads PSUM twice (HW verifier rules).
"""import sys

sys.path.insert(0, "/opt/trn_rl_repo")

import numpy as np

B, S, D = 2, 2048, 2048
H = 16
HD = D // (2 * H)          # 64 per-map head dim
DH = 2 * HD                # 128 per-head dim
HPC = H // 4               # 4 heads per core
NCORES = 8
SCALE = HD ** -0.5         # 0.125
NEG = -8.0e9               # mask add value pre-scale (-1e9 / SCALE)
OUT_MULT = 1.0 - 0.8       # (1 - LBDA_INIT)

TRACE = False
TRACE_DIR = None
LAST_RESULTS = None
LAST_EXEC_NS = None

_PROGRAM_CACHE = {}


def build_program(s=S):
    """Build the per-core Bass program (SPMD: same program, 8 cores)."""
    import concourse.bass as bass
    import concourse.tile as tile
    from concourse import bacc, mybir
    from concourse.bass import ts, ds

    f32 = mybir.dt.float32
    f32r = mybir.dt.float32r
    bf16 = mybir.dt.bfloat16
    AF = mybir.ActivationFunctionType
    OP = mybir.AluOpType

    NCH = s // 512              # token chunks of 512
    KT = s // 128               # k tiles of 128
    KO = D // 128               # contraction chunks over D

    nc = bacc.Bacc()
    xT = nc.declare_dram_parameter("xT", [D, s], f32, isOutput=False)
    xTb = nc.declare_dram_parameter("xTb", [D, s], bf16, isOutput=False)
    wk = nc.declare_dram_parameter("wk", [D, 4 * 128], f32, isOutput=False)
    wq = nc.declare_dram_parameter("wq", [D, 4 * 128], f32, isOutput=False)
    wv = nc.declare_dram_parameter("wv", [D, HPC * DH], bf16, isOutput=False)
    wo = nc.declare_dram_parameter("wo", [HPC * DH, D], bf16, isOutput=False)
    cs = nc.declare_dram_parameter("cs", [128, s], f32, isOutput=False)
    sn = nc.declare_dram_parameter("sn", [128, s], f32, isOutput=False)
    gb = nc.declare_dram_parameter("gb", [128, 2 * HPC], f32, isOutput=False)
    mw = nc.declare_dram_parameter("mw", [128, 256], f32, isOutput=False)
    onem = nc.declare_dram_parameter("onem", [128, 128], bf16, isOutput=False)
    onec = nc.declare_dram_parameter("onec", [128, 128], bf16, isOutput=False)
    onel = nc.declare_dram_parameter("onel", [128, 128], bf16, isOutput=False)
    onecr = nc.declare_dram_parameter("onecr", [128, 128], f32, isOutput=False)
    onelr = nc.declare_dram_parameter("onelr", [128, 128], f32, isOutput=False)
    out = nc.declare_dram_parameter("out", [s, D], f32, isOutput=True)

    r = lambda ap: ap.bitcast(f32r)

    with tile.TileContext(nc) as tc:
        with tc.tile_pool(name="pL", bufs=1) as pL, \
             tc.tile_pool(name="pO", bufs=2) as pO, \
             tc.tile_pool(name="pT", bufs=2) as pT, \
             tc.tile_pool(name="pCS", bufs=2) as pCS, \
             tc.tile_pool(name="pp", bufs=4, space="PSUM") as pp:
            # ---- long-lived SBUF state -----------------------------------
            kt = pL.tile([128, HPC, s], bf16)       # per-head k dims x tokens
            qt = pL.tile([128, HPC, s], bf16)       # per-head q dims x tokens
            vt = pL.tile([128, KT, HPC, DH], bf16)  # k-token x (ki, h, vd)
            wv_sb = pL.tile([128, KO, 512], bf16)
            gb_sb = pL.tile([128, 2 * HPC], f32)
            nc.sync.dma_start(gb_sb[:], gb[:])
            mw_sb = pL.tile([128, 2, 128], f32)
            nc.sync.dma_start(mw_sb[:, :, :], mw[:])
            ones_m = pL.tile([128, 128], bf16)
            nc.sync.dma_start(ones_m[:], onem[:])
            ones_c = pL.tile([128, 128], bf16)
            nc.sync.dma_start(ones_c[:], onec[:])
            ones_l = pL.tile([128, 128], bf16)
            nc.sync.dma_start(ones_l[:], onel[:])
            ones_cr = pL.tile([128, 128], f32r)
            nc.sync.dma_start(ones_cr[:], r(onecr[:]))
            ones_lr = pL.tile([128, 128], f32r)
            nc.sync.dma_start(ones_lr[:], r(onelr[:]))

            def load_x(pool, ncI, tagpfx, engs):
                xth = []
                for hf in range(2):
                    eng = engs[hf]
                    xh = pool.tile([128, KO, 256], f32r, tag="xt", bufs=2,
                                   name=f"x{tagpfx}{ncI}_{hf}")
                    for ko in range(KO):
                        eng.dma_start(
                            xh[:, ko, :],
                            r(xT[ds(ko * 128, 128),
                                 ds(ncI * 512 + hf * 256, 256)]))
                    xth.append(xh)
                return xth

            def load_xv(ncI, engs=(None, None)):
                xth = []
                for hf in range(2):
                    eng = engs[hf] or nc.sync
                    xh = pW2.tile([128, KO, 256], bf16, tag="xv", bufs=2,
                                  name=f"xv{ncI}_{hf}")
                    for ko in range(KO):
                        eng.dma_start(
                            xh[:, ko, :],
                            xTb[ds(ko * 128, 128),
                                ds(ncI * 512 + hf * 256, 256)])
                    xth.append(xh)
                return xth

            def load_cs(ncI, tagpfx, eng=None):
                eng = eng or nc.gpsimd
                csc = pCS.tile([128, 512], f32, tag="cs", name=f"{tagpfx}c{ncI}")
                eng.dma_start(csc[:], cs[:, ts(ncI, 512)])
                snc = pCS.tile([128, 512], f32, tag="sn", name=f"{tagpfx}s{ncI}")
                eng.dma_start(snc[:], sn[:, ts(ncI, 512)])
                return csc[:], snc[:]

            def proj_rope_scatter(w_sb, xth, csc, snc, dst, ncI):
                # two RoPE'd maps -> scatter into dst[dims, head, tokens]
                for pair in (0, 1):
                    be, bo = 2 * pair, 2 * pair + 1
                    pe_ = pp.tile([128, 512], f32, tag="w")
                    po_ = pp.tile([128, 512], f32, tag="w")
                    for hf in range(2):
                        for ko in range(KO):
                            nc.tensor.matmul(pe_[:, ts(hf, 256)],
                                             w_sb[:, ko, ts(be, 128)],
                                             xth[hf][:, ko, :],
                                             start=(ko == 0), stop=(ko == KO - 1))
                        for ko in range(KO):
                            nc.tensor.matmul(po_[:, ts(hf, 256)],
                                             w_sb[:, ko, ts(bo, 128)],
                                             xth[hf][:, ko, :],
                                             start=(ko == 0), stop=(ko == KO - 1))
                    t1 = pT.tile([128, 512], f32, tag="t1")
                    t2 = pT.tile([128, 512], f32, tag="t2")
                    oe = pO.tile([128, 512], bf16, tag="oe", bufs=3)
                    oo = pO.tile([128, 512], bf16, tag="oo", bufs=3)
                    nc.vector.tensor_tensor(t1[:], pe_[:], csc, OP.mult)
                    nc.vector.tensor_tensor(t2[:], po_[:], snc, OP.mult)
                    nc.vector.tensor_tensor(oe[:], t1[:], t2[:], OP.subtract)
                    nc.vector.tensor_tensor(t1[:], pe_[:], snc, OP.mult)
                    nc.vector.tensor_tensor(t2[:], po_[:], csc, OP.mult)
                    nc.vector.tensor_tensor(oo[:], t1[:], t2[:], OP.add)
                    moff = pair * 64
                    for hh in range(HPC):
                        nc.gpsimd.dma_start(dst[ds(moff, 32), hh, ts(ncI, 512)],
                                            oe[ds(hh * 32, 32), :])
                        nc.gpsimd.dma_start(
                            dst[ds(moff + 32, 32), hh, ts(ncI, 512)],
                            oo[ds(hh * 32, 32), :])

            # ---- window 1: k + q projection into SBUF --------------------
            with tc.tile_pool(name="pW1", bufs=1) as pW1:
                wk_sb = pW1.tile([128, KO, 512], f32r)
                wq_sb = pW1.tile([128, KO, 512], f32r)
                for ko in range(KO):
                    nc.scalar.dma_start(wk_sb[:, ko, :], r(wk[ds(ko * 128, 128), :]))

                for ncI in range(NCH):
                    engs = (nc.gpsimd, nc.sync) if ncI == 0 else (nc.scalar, nc.sync)
                    xth = load_x(pW1, ncI, "k", engs)
                    if ncI == 0:
                        for ko in range(KO):
                            eng = nc.sync if ko % 2 == 0 else nc.gpsimd
                            eng.dma_start(wq_sb[:, ko, :],
                                          r(wq[ds(ko * 128, 128), :]))
                    csc, snc = load_cs(ncI, "k",
                                       nc.sync if ncI == 0 else nc.gpsimd)
                    proj_rope_scatter(wk_sb, xth, csc, snc, kt, ncI)
                    if ncI == 0:
                        # warm the Exp table while ACT is otherwise idle
                        warm = pL.tile([1, 4], f32)
                        nc.vector.memset(warm[:], 0.0)
                        nc.scalar.activation(warm[:], warm[:], AF.Exp)
                    proj_rope_scatter(wq_sb, xth, csc, snc, qt, ncI)
                for ko in range(KO):
                    nc.sync.dma_start(wv_sb[:, ko, :], wv[ds(ko * 128, 128), :])

            # ---- window 2: v projection + attention + LN + @W_o ----------
            with tc.tile_pool(name="pW2", bufs=1) as pW2, \
                 tc.tile_pool(name="pA", bufs=3) as pA, \
                 tc.tile_pool(name="pE", bufs=3) as pE, \
                 tc.tile_pool(name="pU", bufs=3) as pU, \
                 tc.tile_pool(name="pST", bufs=4) as pST, \
                 tc.tile_pool(name="pES", bufs=1) as pES:
                wo_sb = pW2.tile([128, HPC, D], bf16)
                for hh in range(HPC):
                    nc.sync.dma_start(wo_sb[:, hh, :], wo[ds(hh * 128, 128), :])

                def emit_po(ncI, att_sb, qis=range(4)):
                    for qi in qis:
                        for nj in range(D // 512):
                            po = pp.tile([128, 512], f32, tag="w")
                            for h in range(HPC):
                                nc.tensor.matmul(
                                    po[:], att_sb[:, h, ts(qi, 128)],
                                    wo_sb[:, h, ts(nj, 512)],
                                    start=(h == 0), stop=(h == HPC - 1))
                            oo = pO.tile([128, 512], f32, tag="po", bufs=4)
                            nc.vector.tensor_copy(out=oo[:], in_=po[:])
                            nc.sync.dma_start(
                                out[ts(ncI * 4 + qi, 128), ts(nj, 512)], oo[:])

                def emit_v_part(ncI, xth, tsub, pv, kos):
                    for ko in kos:
                        nc.tensor.matmul(pv[:],
                                         xth[tsub // 2][:, ko,
                                                        ts(tsub % 2, 128)],
                                         wv_sb[:, ko, :], start=(ko == 0),
                                         stop=(ko == KO - 1))
                    if kos[-1] == KO - 1:
                        ov = pO.tile([128, 512], bf16, tag="ov")
                        nc.vector.tensor_copy(out=ov[:], in_=pv[:])
                        nc.gpsimd.dma_start(vt[:, ncI * 4 + tsub, :, :], ov[:])

                def emit_v(ncI, xth, tsub):
                    pv = pp.tile([128, 512], f32, tag="w")
                    emit_v_part(ncI, xth, tsub, pv, list(range(KO)))

                prev_po = None
                prev_prev_po = None
                xv_next = None
                for ncI in range(NCH):
                    if ncI == 0:
                        xth = load_xv(0)
                        for tsub in range(4):
                            emit_v(0, xth, tsub)
                    xv_next = load_xv(ncI + 1) if ncI + 1 < NCH else None

                    # -- attention row qc = ncI, all heads --
                    att_sb = pA.tile([128, HPC, 512], bf16, tag="att",
                                     name=f"att{ncI}")
                    var_all = pST.tile([128, HPC, 512], f32, tag="var", bufs=1,
                                       name=f"var{ncI}")
                    mu_all = pST.tile([128, HPC, 512], bf16, tag="mu", bufs=1,
                                      name=f"mu{ncI}")
                    klim = 4 * ncI + 4

                    for h in range(HPC):
                        def emit_scores(ki):
                            p = ki - 4 * ncI
                            c0 = 0 if p <= 0 else 128 * p
                            s12 = pp.tile([128, 2, 512], f32, tag="s12",
                                          bufs=2, name=f"s12_{ki}")
                            nc.tensor.matmul(s12[:, 0, c0:],
                                             kt[0:64, h, ts(ki, 128)],
                                             qt[0:64, h, ds(ncI * 512 + c0,
                                                            512 - c0)],
                                             start=True, stop=True,
                                             skip_group_check=True)
                            nc.tensor.matmul(s12[:, 1, c0:],
                                             kt[64:128, h, ts(ki, 128)],
                                             qt[64:128, h, ds(ncI * 512 + c0,
                                                              512 - c0)],
                                             start=True, stop=True,
                                             skip_group_check=True)
                            if p >= 0:
                                co = 128 * p
                                nc.vector.tensor_tensor(
                                    s12[:, :, co:co + 128],
                                    s12[:, :, co:co + 128], mw_sb[:, :, :],
                                    OP.add)
                            e12 = pE.tile([128, 2, 512], bf16, tag="e", bufs=4,
                                          name=f"e12_{ki}")
                            nc.scalar.activation(e12[:, :, c0:], s12[:, :, c0:],
                                                 AF.Exp, scale=SCALE)
                            return e12, c0

                        U1 = pp.tile([128, 512], f32, tag="w")
                        U2 = pp.tile([128, 512], f32, tag="w")
                        esum = pES.tile([128, 2, 512], f32r, tag="es")
                        # filler unit: prev chunk @W_o group, emitted near
                        # the end of the k-loop where the PE drains ahead of
                        # the exp pipeline
                        units = []
                        if h == 0 and prev_prev_po is not None:
                            units.append(
                                lambda: emit_po(ncI - 2, prev_prev_po,
                                                qis=(0,)))
                        elif h > 0 and prev_po is not None:
                            units.append(lambda h=h: emit_po(ncI - 1, prev_po,
                                                             qis=(h,)))
                        es = {0: emit_scores(0)}
                        for ki in range(klim):
                            if ki + 1 < klim:
                                es[ki + 1] = emit_scores(ki + 1)
                            e12, c0 = es.pop(ki)
                            st, sp = (ki == 0), (ki == klim - 1)
                            nc.tensor.matmul(U1[:, c0:], vt[:, ki, h, :],
                                             e12[:, 0, c0:], start=st, stop=sp)
                            nc.tensor.matmul(U2[:, c0:], vt[:, ki, h, :],
                                             e12[:, 1, c0:], start=st, stop=sp)
                            if ki == 0:
                                nc.gpsimd.tensor_copy(out=esum[:, :, :],
                                                      in_=e12[:, :, :])
                            else:
                                nc.gpsimd.tensor_tensor(
                                    esum[:, :, c0:], esum[:, :, c0:],
                                    e12[:, :, c0:], OP.add)
                        while units:
                            units.pop(0)()
                        # denominators from the Pool-accumulated exp sums
                        D1 = pp.tile([128, 512], f32, tag="w")
                        D2 = pp.tile([128, 512], f32, tag="w")
                        nc.tensor.matmul(D1[:], ones_cr[:], esum[:, 0, :],
                                         start=True, stop=True,
                                         skip_group_check=True)
                        nc.tensor.matmul(D2[:], ones_lr[:], esum[:, 1, :],
                                         start=True, stop=True,
                                         skip_group_check=True)
                        if xv_next is not None:
                            emit_v(ncI + 1, xv_next, h)
                        # epilogue: D replicated across partitions already
                        att = att_sb[:, h, :]
                        r1s = pU.tile([128, 512], f32, tag="u")
                        r2s = pU.tile([128, 512], f32, tag="u")
                        nc.vector.reciprocal(out=r1s[:], in_=D1[:])
                        nc.vector.tensor_tensor(r1s[:], U1[:], r1s[:], OP.mult)
                        nc.vector.reciprocal(out=r2s[:], in_=D2[:])
                        nc.vector.tensor_tensor(r2s[:], U2[:], r2s[:], OP.mult)
                        nc.vector.tensor_tensor(att, r1s[:], r2s[:], OP.subtract)
                        # LN stats (raw att): replicated ones-matmuls
                        sq = pU.tile([128, 512], bf16, tag="u")
                        nc.vector.tensor_tensor(sq[:], att, att, OP.mult)
                        MSm = pp.tile([128, 512], f32, tag="w")
                        MSs = pp.tile([128, 512], f32, tag="w")
                        nc.tensor.matmul(MSm[:], ones_m[:], att,
                                         start=True, stop=True,
                                         skip_group_check=True)
                        nc.tensor.matmul(MSs[:], ones_c[:], sq[:],
                                         start=True, stop=True,
                                         skip_group_check=True)
                        nc.vector.tensor_copy(out=mu_all[:, h, :], in_=MSm[:])
                        musq = pU.tile([128, 512], f32, tag="u")
                        nc.gpsimd.tensor_tensor(musq[:], mu_all[:, h, :],
                                                mu_all[:, h, :], OP.mult)
                        nc.vector.scalar_tensor_tensor(
                            var_all[:, h, :], MSs[:], 1.0 / DH, musq[:],
                            op0=OP.mult, op1=OP.subtract)

                    # batched LayerNorm: one eps-add + recip + Sqrt over all
                    # 4 heads (single ACT Sqrt per chunk -> 2 table loads)
                    nc.vector.tensor_scalar(
                        out=var_all[:, :, :], in0=var_all[:, :, :],
                        scalar1=1e-5, scalar2=None, op0=OP.add)
                    nc.vector.reciprocal(out=var_all[:, :, :],
                                         in_=var_all[:, :, :])
                    nc.scalar.activation(var_all[:, :, :], var_all[:, :, :],
                                         AF.Sqrt)
                    for h in range(HPC):
                        att = att_sb[:, h, :]
                        nc.gpsimd.tensor_tensor(mu_all[:, h, :],
                                                mu_all[:, h, :],
                                                var_all[:, h, :], OP.mult)
                        t1 = pU.tile([128, 512], f32, tag="u")
                        nc.gpsimd.tensor_tensor(t1[:], att, var_all[:, h, :],
                                                OP.mult)
                        nc.gpsimd.tensor_tensor(t1[:], t1[:], mu_all[:, h, :],
                                                OP.subtract)
                        nc.gpsimd.tensor_scalar(
                            out=att, in0=t1[:],
                            scalar1=gb_sb[:, h:h + 1],
                            scalar2=gb_sb[:, HPC + h:HPC + h + 1],
                            op0=OP.mult, op1=OP.add)
                    prev_prev_po = prev_po
                    prev_po = att_sb
                emit_po(NCH - 2, prev_prev_po, qis=(0,))
                emit_po(NCH - 1, prev_po)

    nc.finalize()
    return nc


def get_program(s=S):
    if s not in _PROGRAM_CACHE:
        _PROGRAM_CACHE[s] = build_program(s)
    return _PROGRAM_CACHE[s]


def make_core_inputs(x, cos, sin, W_qkv, W_o, ln_gamma, ln_beta, lbda, core, s=S):
    """Host-side shard prep for one core."""
    b, hg = core // 4, core % 4
    heads = list(range(hg * HPC, (hg + 1) * HPC))

    def qk_block_cols(base, dstart):
        # even/odd pair columns for one 32-wide block across the 4 heads
        return [base + hh * DH + dstart + 2 * p for hh in heads for p in range(32)]

    def section(base):
        cols = []
        for dstart in (0, 1, HD, HD + 1):     # m1-even, m1-odd, m2-even, m2-odd
            cols += qk_block_cols(base, dstart)
        return cols

    import ml_dtypes
    bf = ml_dtypes.bfloat16
    wq_ = np.ascontiguousarray(W_qkv[:, section(0)], dtype=np.float32)
    wk_ = np.ascontiguousarray(W_qkv[:, section(D)], dtype=np.float32)
    vcols = [2 * D + hh * DH + dd for hh in heads for dd in range(DH)]
    wv = np.ascontiguousarray(W_qkv[:, vcols].astype(bf))
    worows = [hh * DH + dd for hh in heads for dd in range(DH)]
    wo = np.ascontiguousarray(W_o[worows, :].astype(bf))

    xT = np.ascontiguousarray(x[b].T, dtype=np.float32)
    xTb = np.ascontiguousarray(xT.astype(bf))
    cst = np.ascontiguousarray(np.tile(cos.T, (HPC, 1)), dtype=np.float32)
    snt = np.ascontiguousarray(np.tile(sin.T, (HPC, 1)), dtype=np.float32)

    gb = np.zeros((128, 2 * HPC), dtype=np.float32)
    for j, hh in enumerate(heads):
        gb[:, j] = ln_gamma[hh] * OUT_MULT
        gb[:, HPC + j] = ln_beta[hh] * OUT_MULT

    diag = np.where(np.triu(np.ones((128, 128), dtype=bool)), 0.0, NEG)
    mwide = np.ascontiguousarray(
        np.concatenate([diag, diag], axis=1).astype(np.float32))

    return {
        "xT": xT, "xTb": xTb, "wk": wk_, "wq": wq_, "wv": wv, "wo": wo,
        "cs": cst, "sn": snt, "gb": gb, "mw": mwide,
        "onem": np.full((128, 128), 1.0 / DH, dtype=bf),
        "onec": np.ones((128, 128), dtype=bf),
        "onel": np.full((128, 128), 1.0 / lbda if lbda != 0 else 1e30,
                        dtype=bf),
        "onecr": np.ones((128, 128), dtype=np.float32),
        "onelr": np.full((128, 128), 1.0 / lbda if lbda != 0 else 1e30,
                         dtype=np.float32),
    }


def _mask_is_causal(mask, s=S):
    m = np.asarray(mask).reshape(s, s)
    tril = np.tril(np.ones((s, s), dtype=bool))
    if not np.array_equal(m == 0.0, tril):
        return False
    off = m[~tril]
    return off.size == 0 or (np.all(off <= -1.0e8) and np.all(np.isfinite(off)))


def _numpy_reference(x, mask, cos, sin, W_qkv, W_o, ln_gamma, ln_beta, lbda):
    """Exact-math fallback (used only if the mask is not the causal pattern)."""
    b, s, d = x.shape
    qkv = x @ W_qkv
    q, k, v = np.split(qkv, 3, axis=-1)
    q = q.reshape(b, s, H, DH).transpose(0, 2, 1, 3)
    k = k.reshape(b, s, H, DH).transpose(0, 2, 1, 3)
    v = v.reshape(b, s, H, DH).transpose(0, 2, 1, 3)

    def rope(t):
        tr = t.reshape(b, H, s, HD // 2, 2)
        x1, x2 = tr[..., 0], tr[..., 1]
        c = cos[None, None]
        sn_ = sin[None, None]
        o1 = x1 * c - x2 * sn_
        o2 = x1 * sn_ + x2 * c
        return np.stack([o1, o2], axis=-1).reshape(b, H, s, HD)

    q1, q2 = q[..., :HD], q[..., HD:]
    k1, k2 = k[..., :HD], k[..., HD:]
    q1, k1 = rope(q1), rope(k1)
    q2, k2 = rope(q2), rope(k2)

    def softm(z):
        z = z - z.max(-1, keepdims=True)
        e = np.exp(z)
        return e / e.sum(-1, keepdims=True)

    m = np.asarray(mask).reshape(1, 1, s, s)
    a1 = softm(np.einsum("bhqd,bhkd->bhqk", q1, k1) * SCALE + m)
    a2 = softm(np.einsum("bhqd,bhkd->bhqk", q2, k2) * SCALE + m)
    a = a1 - float(lbda) * a2
    o = np.einsum("bhqk,bhkd->bhqd", a, v)
    mu = o.mean(-1, keepdims=True)
    var = o.var(-1, keepdims=True)
    o = (o - mu) / np.sqrt(var + 1e-5)
    o = o * ln_gamma[None, :, None, :] + ln_beta[None, :, None, :]
    o = o * OUT_MULT
    o = o.transpose(0, 2, 1, 3).reshape(b, s, d)
    return (o @ W_o).astype(np.float32)


def kernel(x, mask, cos, sin, W_qkv, W_o, ln_gamma, ln_beta, lbda):
    global LAST_RESULTS, LAST_EXEC_NS
    x = np.asarray(x, dtype=np.float32)
    cos = np.asarray(cos, dtype=np.float32)
    sin = np.asarray(sin, dtype=np.float32)
    W_qkv = np.asarray(W_qkv, dtype=np.float32)
    W_o = np.asarray(W_o, dtype=np.float32)
    ln_gamma = np.asarray(ln_gamma, dtype=np.float32)
    ln_beta = np.asarray(ln_beta, dtype=np.float32)
    lbda_f = float(np.asarray(lbda))

    if not _mask_is_causal(mask):
        return _numpy_reference(x, mask, cos, sin, W_qkv, W_o,
                                ln_gamma, ln_beta, lbda_f)

    from concourse.bass_utils import run_bass_kernel_spmd

    nc = get_program(S)
    in_maps = [
        make_core_inputs(x, cos, sin, W_qkv, W_o, ln_gamma, ln_beta, lbda_f, c)
        for c in range(NCORES)
    ]
    kwargs = {"trace": TRACE}
    if TRACE and TRACE_DIR:
        kwargs["tmpdir"] = TRACE_DIR
    res = run_bass_kernel_spmd(nc, in_maps, core_ids=list(range(NCORES)),
                               **kwargs)
    LAST_RESULTS = res
    LAST_EXEC_NS = getattr(res, "exec_time_ns", None)

    outf = np.zeros((B, S, D), dtype=np.float32)
    for c in range(NCORES):
        outf[c // 4] += res.results[c]["out"]
    return outf
